# revision 32
# baseline (speedup 1.0000x reference)
"""Swin-style attention (B=64,N=512,C=768,H=12) on 8 TRN2 NeuronCores.

Strategy: pure data-parallel over batch (8 batches/core), no collectives.
Per core, one fused pipeline per batch:
  phase1: qkT = Wqk @ x^T (f32r matmuls), v = x @ Wv^T (natural layout,
          padded with a ones-column per head for fused softmax sums)
  attn:   per head h: sT[j,i] = kT^T@qT (K=64, f32r) -> +biasT (DVE)
          -> exp (ACT, bf16 out) -> oT~[d,i] & sums via [v|1] matmul (bf16)
          -> normalize rows by 1/sums (partition_broadcast + DVE mul)
  proj:   out = oT^T @ Wp^T + pb (bf16 matmul, f32r-grade accuracy not
          needed after softmax averaging)
Scale 1/8 is folded into the q-half of Wqk on the host; softmax runs
without max-subtraction (scores are O(1) by construction).
"""
import sys

sys.path.insert(0, "/opt/trn_rl_repo")
from contextlib import ExitStack

import ml_dtypes
import numpy as np

import concourse.bass as bass
import concourse.mybir as mybir
import concourse.tile as tile
from concourse import bacc
from concourse.bass_utils import run_bass_kernel_spmd
from concourse.masks import make_identity

F32 = mybir.dt.float32
F32R = mybir.dt.float32r
BF16 = mybir.dt.bfloat16

B, N, C, H, HD = 64, 512, 768, 12, 64
NCORES = 8
BL = B // NCORES          # batches per core
T = BL * N                # tokens per core
KC = C // 128             # 6 contraction chunks
NJT = N // 128            # 4 key-side tiles
NIT = N // 128            # 4 query/token tiles
VP = H * (HD + 1)         # 780: v padded with ones column per head
Exp = mybir.ActivationFunctionType.Exp


PB_IS_ZERO = True


def _build():
    nc = bacc.Bacc(target_bir_lowering=False)
    xT_d = nc.dram_tensor("xT", [C, T], BF16, kind="ExternalInput")
    wqk_d = nc.dram_tensor("wqk", [C, 2 * C], BF16, kind="ExternalInput")
    wv_d = nc.dram_tensor("wv", [C, C], BF16, kind="ExternalInput")
    wp_d = nc.dram_tensor("wp", [C, C], BF16, kind="ExternalInput")
    biasT_d = nc.dram_tensor("biasT", [H, NJT, 128, N], BF16, kind="ExternalInput")
    pb_d = nc.dram_tensor("pb", [1, C], F32, kind="ExternalInput")
    out_d = nc.dram_tensor("out", [T, C], F32, kind="ExternalOutput")

    with ExitStack() as ctx:
        tc = ctx.enter_context(tile.TileContext(nc))
        const = ctx.enter_context(tc.tile_pool(name="const", bufs=1))
        perb = ctx.enter_context(tc.tile_pool(name="perb", bufs=2))
        perb1 = ctx.enter_context(tc.tile_pool(name="perb1", bufs=2))
        xt_pool = ctx.enter_context(tc.tile_pool(name="xt", bufs=2))
        pool_p = ctx.enter_context(tc.tile_pool(name="pt", bufs=3))
        pool_r = ctx.enter_context(tc.tile_pool(name="rc", bufs=3))
        pool_o = ctx.enter_context(tc.tile_pool(name="osb", bufs=2))
        dram_p = ctx.enter_context(tc.tile_pool(name="dramp", bufs=2, space="DRAM"))
        mm_ps = ctx.enter_context(tc.tile_pool(name="mmps", bufs=3, space="PSUM"))
        s_ps = ctx.enter_context(tc.tile_pool(name="sps", bufs=1, space="PSUM"))
        o_ps = ctx.enter_context(tc.tile_pool(name="ops", bufs=1, space="PSUM"))

        # ---- constants ----
        wqk = const.tile([128, KC, 2 * C], BF16)
        wv = const.tile([128, KC, C], BF16)
        wp = const.tile([128, KC, C], BF16)
        biasT = const.tile([128, H, NJT, N], BF16)
        pb_bc = const.tile([128, C], F32)
        ident = const.tile([128, 128], BF16)
        make_identity(nc, ident)
        def load_consts_early():
            for kc in range(KC):
                nc.sync.dma_start(
                    out=wqk[:, kc, :], in_=wqk_d[kc * 128:(kc + 1) * 128, :]
                )
            for kc in range(KC):
                nc.sync.dma_start(
                    out=wv[:, kc, :], in_=wv_d[kc * 128:(kc + 1) * 128, :]
                )

        def load_consts_late():
            for kc in range(KC):
                nc.sync.dma_start(
                    out=wp[:, kc, :], in_=wp_d[kc * 128:(kc + 1) * 128, :]
                )
            for h in range(H):
                nc.sync.dma_start(
                    out=biasT[:, h, :, :],
                    in_=biasT_d[h, :, :, :].rearrange("a p b -> p a b"),
                )
            nc.sync.dma_start(out=pb_bc, in_=pb_d[0:1, :].to_broadcast((128, C)))

        def load_x(b):
            xTb = xt_pool.tile([128, KC, N], BF16, tag="xTb")
            for kc in range(KC):
                nc.sync.dma_start(
                    out=xTb[:, kc, :],
                    in_=xT_d[kc * 128:(kc + 1) * 128, b * N:(b + 1) * N],
                )
            return xTb

        def alloc_qkT():
            return perb.tile([128, 2 * H // 2, N], BF16, tag="qkT", name="qkT")

        def alloc_vpad():
            v_pad = perb.tile([128, NIT, VP], BF16, tag="v_pad")
            ones_view = v_pad.rearrange("p a (h e) -> p (a h) e", e=HD + 1)
            nc.vector.memset(ones_view[:, :, HD:HD + 1], 1.0)
            return v_pad

        def qk_tile(qkT, xTb, rt):
            ps = mm_ps.tile([128, N], F32, tag="mm")
            for kc in range(KC):
                nc.tensor.matmul(
                    ps,
                    wqk[:, kc, rt * 128:(rt + 1) * 128],
                    xTb[:, kc, :],
                    start=(kc == 0),
                    stop=(kc == KC - 1),
                )
            nc.vector.tensor_copy(out=qkT[:, rt, :], in_=ps)

        def v_tile(v_pad, xTb, it, nh):
            ps = mm_ps.tile([128, C // 2], F32, tag="mm")
            for kc in range(KC):
                nc.tensor.matmul(
                    ps,
                    xTb[:, kc, it * 128:(it + 1) * 128],
                    wv[:, kc, nh * 384:(nh + 1) * 384],
                    start=(kc == 0),
                    stop=(kc == KC - 1),
                )
            dest = v_pad[:, it, :].rearrange("p (h e) -> p h e", e=HD + 1)
            nc.scalar.copy(
                out=dest[:, nh * 6:(nh + 1) * 6, 0:HD],
                in_=ps.rearrange("p (h e) -> p h e", e=HD),
            )

        def proj_tile(b, oT, it):
            outsb = pool_o.tile([128, C], F32, tag="outsb")
            for ct in range(2):
                ps = mm_ps.tile([128, C // 2], F32, tag="mm")
                for kc in range(KC):
                    nc.tensor.matmul(
                        ps,
                        oT[:, kc, it * 128:(it + 1) * 128],
                        wp[:, kc, ct * 384:(ct + 1) * 384],
                        start=(kc == 0),
                        stop=(kc == KC - 1),
                    )
                if PB_IS_ZERO:
                    nc.scalar.copy(out=outsb[:, ct * 384:(ct + 1) * 384], in_=ps)
                else:
                    nc.vector.tensor_add(
                        outsb[:, ct * 384:(ct + 1) * 384],
                        ps,
                        pb_bc[:, ct * 384:(ct + 1) * 384],
                    )
            nc.sync.dma_start(
                out=out_d[b * N + it * 128: b * N + (it + 1) * 128, :],
                in_=outsb,
            )

        def pair_scores(qkT, hp, jp):
            # head pair (hA even -> PE rows 0-63, hB odd -> rows 64-127):
            # their K=64 score matmuls execute concurrently on disjoint
            # row groups of the systolic array; I^T@biasT rides the same
            # PSUM accumulation; exp straight from PSUM, 1024 wide.
            rq, rk = hp, H // 2 + hp
            psA = s_ps.tile([128, 2, N], F32, tag="sTA")
            psB = s_ps.tile([128, 2, N], F32, tag="sTB")
            for jl in range(2):
                jt = jp * 2 + jl
                nc.tensor.matmul(
                    psA[:, jl, :],
                    qkT[0:64, rk, jt * 128:(jt + 1) * 128],
                    qkT[0:64, rq, :],
                    start=True,
                    stop=False,
                )
                nc.tensor.matmul(
                    psB[:, jl, :],
                    qkT[64:128, rk, jt * 128:(jt + 1) * 128],
                    qkT[64:128, rq, :],
                    start=True,
                    stop=False,
                )
            for jl in range(2):
                jt = jp * 2 + jl
                nc.tensor.matmul(
                    psA[:, jl, :], ident, biasT[:, 2 * hp, jt, :],
                    start=False, stop=True,
                )
                nc.tensor.matmul(
                    psB[:, jl, :], ident, biasT[:, 2 * hp + 1, jt, :],
                    start=False, stop=True,
                )
            ptA = pool_p.tile([128, 2, N], BF16, tag="pTA")
            nc.scalar.activation(out=ptA, in_=psA, func=Exp)
            ptB = pool_p.tile([128, 2, N], BF16, tag="pTB")
            nc.scalar.activation(out=ptB, in_=psB, func=Exp)
            return ptA, ptB

        def head_out(oT, v_pad, h, pts):
            po = (h % 2) * 64
            rqo = h // 2
            pso = o_ps.tile([HD + 1, N], F32, tag="oT")
            for jt in range(NJT):
                vp = v_pad[:, jt, :].rearrange("p (h e) -> p h e", e=HD + 1)
                nc.tensor.matmul(
                    pso,
                    vp[:, h, :],
                    pts[jt // 2][:, jt % 2, :],
                    start=(jt == 0),
                    stop=(jt == NJT - 1),
                )
            nc.vector.tensor_copy(out=oT[po:po + 64, rqo, :], in_=pso[0:HD, :])
            smc = pool_r.tile([65, N], F32, tag="smc")
            nc.vector.tensor_copy(out=smc[HD:HD + 1, :], in_=pso[HD:HD + 1, :])
            rcd = dram_p.tile([1, N], F32, tag="rcd")
            nc.sync.dma_start(out=rcd, in_=smc[HD:HD + 1, :])
            return rcd

        def pair_norm(oT, hp, rcdA, rcdB):
            # one [128,N] tile: 1/sums(A) on partitions 0-63, 1/sums(B) on
            # 64-127 -> a single reciprocal + two in-place multiplies.
            rqo = hp
            rcf = pool_r.tile([128, N], F32, tag="rcf")
            nc.sync.dma_start(out=rcf[0:64, :], in_=rcdA[0:1, :].to_broadcast((64, N)))
            nc.sync.dma_start(out=rcf[64:128, :], in_=rcdB[0:1, :].to_broadcast((64, N)))
            nc.vector.reciprocal_approx_fast(out=rcf, in_=rcf)
            nc.gpsimd.tensor_mul(
                oT[0:64, rqo, :], oT[0:64, rqo, :], rcf[0:64, :]
            )
            nc.gpsimd.tensor_mul(
                oT[64:128, rqo, :], oT[64:128, rqo, :], rcf[64:128, :]
            )

        # ---- software-pipelined schedule ----
        # A persistent filler queue carries the next batch's qkv tiles and
        # the previous batch's projection tiles; pairs pop fillers between
        # their score and o matmuls, and dependencies are force-drained
        # just in time.  This keeps the PE dense through the last batch.
        work = []  # list of (key, fn); key=(kind, b, a, c)

        def drain(pred):
            i = 0
            while i < len(work):
                if pred(work[i][0]):
                    work.pop(i)[1]()
                else:
                    i += 1

        def pop_one():
            if work:
                work.pop(0)[1]()

        # startup: interleave wqk/x(0) chunk loads so the first matmul can
        # begin after one chunk of each.
        xT_cur = xt_pool.tile([128, KC, N], BF16, tag="xTb", name="xT0")
        for kc in range(KC):
            nc.sync.dma_start(
                out=wqk[:, kc, :], in_=wqk_d[kc * 128:(kc + 1) * 128, :]
            )
            nc.sync.dma_start(
                out=xT_cur[:, kc, :], in_=xT_d[kc * 128:(kc + 1) * 128, 0:N]
            )
        for kc in range(KC):
            nc.sync.dma_start(out=wv[:, kc, :], in_=wv_d[kc * 128:(kc + 1) * 128, :])
        qkT_cur = alloc_qkT()
        vp_cur = alloc_vpad()
        for rt in range(12):
            qk_tile(qkT_cur, xT_cur, rt)
        load_consts_late()
        for it in range(NIT):
            for nh in range(2):
                v_tile(vp_cur, xT_cur, it, nh)

        oT_prev, b_prev = None, None
        for b in range(BL):
            qkT, v_pad = qkT_cur, vp_cur
            if oT_prev is not None:
                ob, oprev = b_prev, oT_prev
                work.extend(
                    (("proj", ob, it, 0), (lambda it=it, ob=ob, ot=oprev:
                                           proj_tile(ob, ot, it)))
                    for it in range(NIT)
                )
            if b + 1 < BL:
                xT_nxt = load_x(b + 1)
                qkT_cur = alloc_qkT()
                vp_cur = alloc_vpad()
                nb = b + 1
                work.extend(
                    (("qk", nb, rt, 0), (lambda rt=rt, q=qkT_cur, x=xT_nxt:
                                         qk_tile(q, x, rt)))
                    for rt in range(12)
                )
                work.extend(
                    (("v", nb, nh, it), (lambda it=it, nh=nh, v=vp_cur,
                                         x=xT_nxt: v_tile(v, x, it, nh)))
                    for nh in range(2) for it in range(NIT)
                )
            oT = perb1.tile([128, KC, N], BF16, tag="oT")
            for hp in range(H // 2):
                nh_need = (2 * hp) // 6
                drain(lambda k: k[0] == "qk" and k[1] == b
                      and k[2] in (hp, H // 2 + hp))
                pts01 = pair_scores(qkT, hp, 0)
                pop_one()
                pts23 = pair_scores(qkT, hp, 1)
                pop_one()
                drain(lambda k: k[0] == "v" and k[1] == b and k[2] == nh_need)
                ptsA = [pts01[0], pts23[0]]
                ptsB = [pts01[1], pts23[1]]
                rcdA = head_out(oT, v_pad, 2 * hp, ptsA)
                rcdB = head_out(oT, v_pad, 2 * hp + 1, ptsB)
                pair_norm(oT, hp, rcdA, rcdB)
            # anything of this batch's own phase1 left must go now; stale
            # proj must clear before its oT slot is re-used next batch.
            drain(lambda k: k[1] <= b and k[0] != "proj")
            drain(lambda k: k[0] == "proj" and k[1] < b)
            oT_prev, b_prev = oT, b
        drain(lambda k: True)
        for it in range(NIT):
            proj_tile(b_prev, oT_prev, it)
    nc.finalize()
    return nc


def kernel(x, qkv_w, proj_w, proj_b, bias_table, _trace=False, _tmpdir=None):
    x = np.asarray(x, dtype=np.float32)
    qkv_w = np.asarray(qkv_w, dtype=np.float32)
    proj_w = np.asarray(proj_w, dtype=np.float32)
    proj_b = np.asarray(proj_b, dtype=np.float32)
    bias_table = np.asarray(bias_table, dtype=np.float32)

    # host-side layout prep (weights + bias table expansion)
    wq_scaled = qkv_w.copy()
    wq_scaled[:C] *= HD ** (-0.5)
    wqk = np.ascontiguousarray(wq_scaled[: 2 * C].T).astype(ml_dtypes.bfloat16)
    wv = np.ascontiguousarray(qkv_w[2 * C:].T).astype(ml_dtypes.bfloat16)
    wp = np.ascontiguousarray(proj_w.T).astype(ml_dtypes.bfloat16)
    ii = np.arange(N)
    idx = ii[None, :] - ii[:, None] + (N - 1)                     # [j, i]
    biasT = np.ascontiguousarray(
        bias_table[idx].transpose(2, 0, 1).reshape(H, NJT, 128, N)
    ).astype(ml_dtypes.bfloat16)
    pb = proj_b.reshape(1, C)

    global PB_IS_ZERO
    PB_IS_ZERO = not np.any(proj_b)
    nc = _build()
    in_maps = []
    for m in range(NCORES):
        xs = x[m * BL:(m + 1) * BL]                               # [8, 512, 768]
        xT = np.ascontiguousarray(xs.transpose(2, 0, 1).reshape(C, T)).astype(ml_dtypes.bfloat16)
        in_maps.append(
            {"xT": xT, "wqk": wqk, "wv": wv, "wp": wp, "biasT": biasT, "pb": pb}
        )
    res = run_bass_kernel_spmd(
        nc, in_maps, core_ids=list(range(NCORES)), trace=_trace, tmpdir=_tmpdir
    )
    out = np.concatenate(
        [res.results[m]["out"].reshape(BL, N, C) for m in range(NCORES)], axis=0
    )
    if _trace:
        return out, res
    return out


# revision 33
# speedup vs baseline: 1.0530x; 1.0530x over previous
"""Swin-style attention (B=64,N=512,C=768,H=12) on 8 TRN2 NeuronCores.

Strategy: pure data-parallel over batch (8 batches/core), no collectives.
Per core, one fused pipeline per batch:
  phase1: qkT = Wqk @ x^T (f32r matmuls), v = x @ Wv^T (natural layout,
          padded with a ones-column per head for fused softmax sums)
  attn:   per head h: sT[j,i] = kT^T@qT (K=64, f32r) -> +biasT (DVE)
          -> exp (ACT, bf16 out) -> oT~[d,i] & sums via [v|1] matmul (bf16)
          -> normalize rows by 1/sums (partition_broadcast + DVE mul)
  proj:   out = oT^T @ Wp^T + pb (bf16 matmul, f32r-grade accuracy not
          needed after softmax averaging)
Scale 1/8 is folded into the q-half of Wqk on the host; softmax runs
without max-subtraction (scores are O(1) by construction).
"""
import sys

sys.path.insert(0, "/opt/trn_rl_repo")
from contextlib import ExitStack

import ml_dtypes
import numpy as np

import concourse.bass as bass
import concourse.mybir as mybir
import concourse.tile as tile
from concourse import bacc
from concourse.bass_utils import run_bass_kernel_spmd
from concourse.masks import make_identity

F32 = mybir.dt.float32
F32R = mybir.dt.float32r
BF16 = mybir.dt.bfloat16

B, N, C, H, HD = 64, 512, 768, 12, 64
NCORES = 8
BL = B // NCORES          # batches per core
T = BL * N                # tokens per core
KC = C // 128             # 6 contraction chunks
NJT = N // 128            # 4 key-side tiles
NIT = N // 128            # 4 query/token tiles
VP = H * (HD + 1)         # 780: v padded with ones column per head
Exp = mybir.ActivationFunctionType.Exp


PB_IS_ZERO = True


def _build():
    nc = bacc.Bacc(target_bir_lowering=False)
    xT_d = nc.dram_tensor("xT", [C, T], BF16, kind="ExternalInput")
    wqk_d = nc.dram_tensor("wqk", [C, 2 * C], BF16, kind="ExternalInput")
    wv_d = nc.dram_tensor("wv", [C, C], BF16, kind="ExternalInput")
    wp_d = nc.dram_tensor("wp", [C, C], BF16, kind="ExternalInput")
    biasT_d = nc.dram_tensor("biasT", [H, NJT, 128, N], BF16, kind="ExternalInput")
    pb_d = nc.dram_tensor("pb", [1, C], F32, kind="ExternalInput")
    out_d = nc.dram_tensor("out", [T, C], F32, kind="ExternalOutput")

    with ExitStack() as ctx:
        tc = ctx.enter_context(tile.TileContext(nc))
        const = ctx.enter_context(tc.tile_pool(name="const", bufs=1))
        perb = ctx.enter_context(tc.tile_pool(name="perb", bufs=2))
        perb1 = ctx.enter_context(tc.tile_pool(name="perb1", bufs=2))
        xt_pool = ctx.enter_context(tc.tile_pool(name="xt", bufs=2))
        pool_p = ctx.enter_context(tc.tile_pool(name="pt", bufs=3))
        pool_r = ctx.enter_context(tc.tile_pool(name="rc", bufs=3))
        pool_o = ctx.enter_context(tc.tile_pool(name="osb", bufs=2))
        dram_p = ctx.enter_context(tc.tile_pool(name="dramp", bufs=2, space="DRAM"))
        mm_ps = ctx.enter_context(tc.tile_pool(name="mmps", bufs=3, space="PSUM"))
        s_ps = ctx.enter_context(tc.tile_pool(name="sps", bufs=1, space="PSUM"))
        o_ps = ctx.enter_context(tc.tile_pool(name="ops", bufs=1, space="PSUM"))

        # ---- constants ----
        wqk = const.tile([128, KC, 2 * C], BF16)
        wv = const.tile([128, KC, C], BF16)
        wp = const.tile([128, KC, C], BF16)
        biasT = const.tile([128, H, NJT, N], BF16)
        pb_bc = const.tile([128, C], F32)
        ident = const.tile([128, 128], BF16)
        make_identity(nc, ident)
        def load_consts_early():
            for kc in range(KC):
                nc.sync.dma_start(
                    out=wqk[:, kc, :], in_=wqk_d[kc * 128:(kc + 1) * 128, :]
                )
            for kc in range(KC):
                nc.sync.dma_start(
                    out=wv[:, kc, :], in_=wv_d[kc * 128:(kc + 1) * 128, :]
                )

        def load_consts_late():
            for kc in range(KC):
                nc.sync.dma_start(
                    out=wp[:, kc, :], in_=wp_d[kc * 128:(kc + 1) * 128, :]
                )
            for h in range(H):
                nc.sync.dma_start(
                    out=biasT[:, h, :, :],
                    in_=biasT_d[h, :, :, :].rearrange("a p b -> p a b"),
                )
            nc.sync.dma_start(out=pb_bc, in_=pb_d[0:1, :].to_broadcast((128, C)))

        def load_x(b):
            xTb = xt_pool.tile([128, KC, N], BF16, tag="xTb")
            for kc in range(KC):
                nc.sync.dma_start(
                    out=xTb[:, kc, :],
                    in_=xT_d[kc * 128:(kc + 1) * 128, b * N:(b + 1) * N],
                )
            return xTb

        def alloc_qkT():
            return perb.tile([128, 2 * H // 2, N], BF16, tag="qkT", name="qkT")

        def alloc_vpad():
            v_pad = perb.tile([128, NIT, VP], BF16, tag="v_pad")
            ones_view = v_pad.rearrange("p a (h e) -> p (a h) e", e=HD + 1)
            nc.vector.memset(ones_view[:, :, HD:HD + 1], 1.0)
            return v_pad

        def qk_tile(qkT, xTb, rt):
            ps = mm_ps.tile([128, N], F32, tag="mm")
            for kc in range(KC):
                nc.tensor.matmul(
                    ps,
                    wqk[:, kc, rt * 128:(rt + 1) * 128],
                    xTb[:, kc, :],
                    start=(kc == 0),
                    stop=(kc == KC - 1),
                )
            nc.vector.tensor_copy(out=qkT[:, rt, :], in_=ps)

        def v_tile(v_pad, xTb, it, nh):
            ps = mm_ps.tile([128, C // 2], F32, tag="mm")
            for kc in range(KC):
                nc.tensor.matmul(
                    ps,
                    xTb[:, kc, it * 128:(it + 1) * 128],
                    wv[:, kc, nh * 384:(nh + 1) * 384],
                    start=(kc == 0),
                    stop=(kc == KC - 1),
                )
            dest = v_pad[:, it, :].rearrange("p (h e) -> p h e", e=HD + 1)
            nc.scalar.copy(
                out=dest[:, nh * 6:(nh + 1) * 6, 0:HD],
                in_=ps.rearrange("p (h e) -> p h e", e=HD),
            )

        def proj_tile(b, oT, it):
            outsb = pool_o.tile([128, C], F32, tag="outsb")
            for ct in range(2):
                ps = mm_ps.tile([128, C // 2], F32, tag="mm")
                for kc in range(KC):
                    nc.tensor.matmul(
                        ps,
                        oT[:, kc, it * 128:(it + 1) * 128],
                        wp[:, kc, ct * 384:(ct + 1) * 384],
                        start=(kc == 0),
                        stop=(kc == KC - 1),
                    )
                if PB_IS_ZERO:
                    nc.scalar.copy(out=outsb[:, ct * 384:(ct + 1) * 384], in_=ps)
                else:
                    nc.vector.tensor_add(
                        outsb[:, ct * 384:(ct + 1) * 384],
                        ps,
                        pb_bc[:, ct * 384:(ct + 1) * 384],
                    )
            nc.sync.dma_start(
                out=out_d[b * N + it * 128: b * N + (it + 1) * 128, :],
                in_=outsb,
            )

        def pair_scores(qkT, hp, jp):
            # head pair (hA even -> PE rows 0-63, hB odd -> rows 64-127):
            # their K=64 score matmuls execute concurrently on disjoint
            # row groups of the systolic array; I^T@biasT rides the same
            # PSUM accumulation; exp straight from PSUM, 1024 wide.
            rq, rk = hp, H // 2 + hp
            psA = s_ps.tile([128, 2, N], F32, tag="sTA")
            psB = s_ps.tile([128, 2, N], F32, tag="sTB")
            for jl in range(2):
                jt = jp * 2 + jl
                nc.tensor.matmul(
                    psA[:, jl, :],
                    qkT[0:64, rk, jt * 128:(jt + 1) * 128],
                    qkT[0:64, rq, :],
                    start=True,
                    stop=False,
                )
                nc.tensor.matmul(
                    psB[:, jl, :],
                    qkT[64:128, rk, jt * 128:(jt + 1) * 128],
                    qkT[64:128, rq, :],
                    start=True,
                    stop=False,
                )
            for jl in range(2):
                jt = jp * 2 + jl
                nc.tensor.matmul(
                    psA[:, jl, :], ident, biasT[:, 2 * hp, jt, :],
                    start=False, stop=True,
                )
                nc.tensor.matmul(
                    psB[:, jl, :], ident, biasT[:, 2 * hp + 1, jt, :],
                    start=False, stop=True,
                )
            ptA = pool_p.tile([128, 2, N], BF16, tag="pTA")
            nc.scalar.activation(out=ptA, in_=psA, func=Exp)
            ptB = pool_p.tile([128, 2, N], BF16, tag="pTB")
            nc.scalar.activation(out=ptB, in_=psB, func=Exp)
            return ptA, ptB

        def head_out(oT, v_pad, h, pts):
            po = (h % 2) * 64
            rqo = h // 2
            pso = o_ps.tile([HD + 1, N], F32, tag="oT")
            for jt in range(NJT):
                vp = v_pad[:, jt, :].rearrange("p (h e) -> p h e", e=HD + 1)
                nc.tensor.matmul(
                    pso,
                    vp[:, h, :],
                    pts[jt // 2][:, jt % 2, :],
                    start=(jt == 0),
                    stop=(jt == NJT - 1),
                )
            nc.vector.tensor_copy(out=oT[po:po + 64, rqo, :], in_=pso[0:HD, :])
            smc = pool_r.tile([65, N], F32, tag="smc")
            nc.vector.tensor_copy(out=smc[HD:HD + 1, :], in_=pso[HD:HD + 1, :])
            rcd = dram_p.tile([1, N], F32, tag="rcd")
            nc.sync.dma_start(out=rcd, in_=smc[HD:HD + 1, :])
            return rcd

        def pair_norm(oT, hp, rcdA, rcdB):
            # one [128,N] tile: 1/sums(A) on partitions 0-63, 1/sums(B) on
            # 64-127 -> a single reciprocal + two in-place multiplies.
            rqo = hp
            rcf = pool_r.tile([128, N], F32, tag="rcf")
            nc.sync.dma_start(out=rcf[0:64, :], in_=rcdA[0:1, :].to_broadcast((64, N)))
            nc.sync.dma_start(out=rcf[64:128, :], in_=rcdB[0:1, :].to_broadcast((64, N)))
            nc.vector.reciprocal_approx_fast(out=rcf, in_=rcf)
            nc.gpsimd.tensor_mul(
                oT[0:64, rqo, :], oT[0:64, rqo, :], rcf[0:64, :]
            )
            nc.gpsimd.tensor_mul(
                oT[64:128, rqo, :], oT[64:128, rqo, :], rcf[64:128, :]
            )

        # ---- software-pipelined schedule ----
        # A persistent filler queue carries the next batch's qkv tiles and
        # the previous batch's projection tiles; pairs pop fillers between
        # their score and o matmuls, and dependencies are force-drained
        # just in time.  This keeps the PE dense through the last batch.
        work = []  # list of (key, fn); key=(kind, b, a, c)

        def drain(pred):
            i = 0
            while i < len(work):
                if pred(work[i][0]):
                    work.pop(i)[1]()
                else:
                    i += 1

        def pop_one():
            if work:
                work.pop(0)[1]()

        # startup: interleave wqk/x(0) chunk loads so the first matmul can
        # begin after one chunk of each.
        xT_cur = xt_pool.tile([128, KC, N], BF16, tag="xTb", name="xT0")
        for kc in range(KC):
            nc.sync.dma_start(
                out=wqk[:, kc, :], in_=wqk_d[kc * 128:(kc + 1) * 128, :]
            )
            nc.sync.dma_start(
                out=xT_cur[:, kc, :], in_=xT_d[kc * 128:(kc + 1) * 128, 0:N]
            )
        for kc in range(KC):
            nc.sync.dma_start(out=wv[:, kc, :], in_=wv_d[kc * 128:(kc + 1) * 128, :])
        qkT_cur = alloc_qkT()
        vp_cur = alloc_vpad()
        for rt in range(12):
            qk_tile(qkT_cur, xT_cur, rt)
        load_consts_late()
        for it in range(NIT):
            for nh in range(2):
                v_tile(vp_cur, xT_cur, it, nh)

        oT_prev, b_prev = None, None
        for b in range(BL):
            qkT, v_pad = qkT_cur, vp_cur
            fillers = []
            if b + 1 < BL:
                xT_nxt = load_x(b + 1)
                qkT_cur = alloc_qkT()
                vp_cur = alloc_vpad()
                fillers += [
                    (lambda rt=rt, q=qkT_cur, x=xT_nxt: qk_tile(q, x, rt))
                    for rt in range(12)
                ]
                fillers += [
                    (lambda it=it, nh=nh, v=vp_cur, x=xT_nxt:
                     v_tile(v, x, it, nh))
                    for it in range(NIT) for nh in range(2)
                ]
            if oT_prev is not None:
                fillers += [
                    (lambda it=it, ob=b_prev, ot=oT_prev: proj_tile(ob, ot, it))
                    for it in range(NIT)
                ]
            fi = 0
            oT = perb1.tile([128, KC, N], BF16, tag="oT")
            for hp in range(H // 2):
                pts01 = pair_scores(qkT, hp, 0)
                if fi < len(fillers):
                    fillers[fi](); fi += 1
                pts23 = pair_scores(qkT, hp, 1)
                if fi < len(fillers):
                    fillers[fi](); fi += 1
                ptsA = [pts01[0], pts23[0]]
                ptsB = [pts01[1], pts23[1]]
                rcdA = head_out(oT, v_pad, 2 * hp, ptsA)
                rcdB = head_out(oT, v_pad, 2 * hp + 1, ptsB)
                pair_norm(oT, hp, rcdA, rcdB)
            for f in fillers[fi:]:
                f()
            oT_prev, b_prev = oT, b
        for it in range(NIT):
            proj_tile(b_prev, oT_prev, it)
    nc.finalize()
    return nc


def kernel(x, qkv_w, proj_w, proj_b, bias_table, _trace=False, _tmpdir=None):
    x = np.asarray(x, dtype=np.float32)
    qkv_w = np.asarray(qkv_w, dtype=np.float32)
    proj_w = np.asarray(proj_w, dtype=np.float32)
    proj_b = np.asarray(proj_b, dtype=np.float32)
    bias_table = np.asarray(bias_table, dtype=np.float32)

    # host-side layout prep (weights + bias table expansion)
    wq_scaled = qkv_w.copy()
    wq_scaled[:C] *= HD ** (-0.5)
    wqk = np.ascontiguousarray(wq_scaled[: 2 * C].T).astype(ml_dtypes.bfloat16)
    wv = np.ascontiguousarray(qkv_w[2 * C:].T).astype(ml_dtypes.bfloat16)
    wp = np.ascontiguousarray(proj_w.T).astype(ml_dtypes.bfloat16)
    ii = np.arange(N)
    idx = ii[None, :] - ii[:, None] + (N - 1)                     # [j, i]
    biasT = np.ascontiguousarray(
        bias_table[idx].transpose(2, 0, 1).reshape(H, NJT, 128, N)
    ).astype(ml_dtypes.bfloat16)
    pb = proj_b.reshape(1, C)

    global PB_IS_ZERO
    PB_IS_ZERO = not np.any(proj_b)
    nc = _build()
    in_maps = []
    for m in range(NCORES):
        xs = x[m * BL:(m + 1) * BL]                               # [8, 512, 768]
        xT = np.ascontiguousarray(xs.transpose(2, 0, 1).reshape(C, T)).astype(ml_dtypes.bfloat16)
        in_maps.append(
            {"xT": xT, "wqk": wqk, "wv": wv, "wp": wp, "biasT": biasT, "pb": pb}
        )
    res = run_bass_kernel_spmd(
        nc, in_maps, core_ids=list(range(NCORES)), trace=_trace, tmpdir=_tmpdir
    )
    out = np.concatenate(
        [res.results[m]["out"].reshape(BL, N, C) for m in range(NCORES)], axis=0
    )
    if _trace:
        return out, res
    return out


# revision 34
# speedup vs baseline: 1.0570x; 1.0038x over previous
"""Swin-style attention (B=64,N=512,C=768,H=12) on 8 TRN2 NeuronCores.

Strategy: pure data-parallel over batch (8 batches/core), no collectives.
Per core, one fused pipeline per batch:
  phase1: qkT = Wqk @ x^T (f32r matmuls), v = x @ Wv^T (natural layout,
          padded with a ones-column per head for fused softmax sums)
  attn:   per head h: sT[j,i] = kT^T@qT (K=64, f32r) -> +biasT (DVE)
          -> exp (ACT, bf16 out) -> oT~[d,i] & sums via [v|1] matmul (bf16)
          -> normalize rows by 1/sums (partition_broadcast + DVE mul)
  proj:   out = oT^T @ Wp^T + pb (bf16 matmul, f32r-grade accuracy not
          needed after softmax averaging)
Scale 1/8 is folded into the q-half of Wqk on the host; softmax runs
without max-subtraction (scores are O(1) by construction).
"""
import sys

sys.path.insert(0, "/opt/trn_rl_repo")
from contextlib import ExitStack

import ml_dtypes
import numpy as np

import concourse.bass as bass
import concourse.mybir as mybir
import concourse.tile as tile
from concourse import bacc
from concourse.bass_utils import run_bass_kernel_spmd
from concourse.masks import make_identity

F32 = mybir.dt.float32
F32R = mybir.dt.float32r
BF16 = mybir.dt.bfloat16

B, N, C, H, HD = 64, 512, 768, 12, 64
NCORES = 8
BL = B // NCORES          # batches per core
T = BL * N                # tokens per core
KC = C // 128             # 6 contraction chunks
NJT = N // 128            # 4 key-side tiles
NIT = N // 128            # 4 query/token tiles
VP = H * (HD + 1)         # 780: v padded with ones column per head
Exp = mybir.ActivationFunctionType.Exp


PB_IS_ZERO = True


def _build():
    nc = bacc.Bacc(target_bir_lowering=False)
    xT_d = nc.dram_tensor("xT", [C, T], BF16, kind="ExternalInput")
    wqk_d = nc.dram_tensor("wqk", [C, 2 * C], BF16, kind="ExternalInput")
    wv_d = nc.dram_tensor("wv", [C, C], BF16, kind="ExternalInput")
    wp_d = nc.dram_tensor("wp", [C, C], BF16, kind="ExternalInput")
    biasT_d = nc.dram_tensor("biasT", [H, NJT, 128, N], BF16, kind="ExternalInput")
    pb_d = nc.dram_tensor("pb", [1, C], F32, kind="ExternalInput")
    out_d = nc.dram_tensor("out", [T, C], F32, kind="ExternalOutput")

    with ExitStack() as ctx:
        tc = ctx.enter_context(tile.TileContext(nc))
        const = ctx.enter_context(tc.tile_pool(name="const", bufs=1))
        perb = ctx.enter_context(tc.tile_pool(name="perb", bufs=2))
        perb1 = ctx.enter_context(tc.tile_pool(name="perb1", bufs=2))
        xt_pool = ctx.enter_context(tc.tile_pool(name="xt", bufs=2))
        pool_p = ctx.enter_context(tc.tile_pool(name="pt", bufs=3))
        pool_r = ctx.enter_context(tc.tile_pool(name="rc", bufs=3))
        pool_o = ctx.enter_context(tc.tile_pool(name="osb", bufs=2))
        dram_p = ctx.enter_context(tc.tile_pool(name="dramp", bufs=2, space="DRAM"))
        mm_ps = ctx.enter_context(tc.tile_pool(name="mmps", bufs=3, space="PSUM"))
        s_ps = ctx.enter_context(tc.tile_pool(name="sps", bufs=1, space="PSUM"))
        o_ps = ctx.enter_context(tc.tile_pool(name="ops", bufs=1, space="PSUM"))

        # ---- constants ----
        wqk = const.tile([128, KC, 2 * C], BF16)
        wv = const.tile([128, KC, C], BF16)
        wp = const.tile([128, KC, C], BF16)
        biasT = const.tile([128, H, NJT, N], BF16)
        pb_bc = const.tile([128, C], F32)
        ident = const.tile([128, 128], BF16)
        make_identity(nc, ident)
        def load_consts_early():
            for kc in range(KC):
                nc.sync.dma_start(
                    out=wqk[:, kc, :], in_=wqk_d[kc * 128:(kc + 1) * 128, :]
                )
            for kc in range(KC):
                nc.sync.dma_start(
                    out=wv[:, kc, :], in_=wv_d[kc * 128:(kc + 1) * 128, :]
                )

        def load_consts_late():
            for kc in range(KC):
                nc.sync.dma_start(
                    out=wp[:, kc, :], in_=wp_d[kc * 128:(kc + 1) * 128, :]
                )
            for h in range(H):
                nc.sync.dma_start(
                    out=biasT[:, h, :, :],
                    in_=biasT_d[h, :, :, :].rearrange("a p b -> p a b"),
                )
            nc.sync.dma_start(out=pb_bc, in_=pb_d[0:1, :].to_broadcast((128, C)))

        def load_x(b):
            xTb = xt_pool.tile([128, KC, N], BF16, tag="xTb")
            for kc in range(KC):
                nc.sync.dma_start(
                    out=xTb[:, kc, :],
                    in_=xT_d[kc * 128:(kc + 1) * 128, b * N:(b + 1) * N],
                )
            return xTb

        def alloc_qkT():
            return perb.tile([128, 2 * H // 2, N], BF16, tag="qkT", name="qkT")

        def alloc_vpad():
            v_pad = perb.tile([128, NIT, VP], BF16, tag="v_pad")
            ones_view = v_pad.rearrange("p a (h e) -> p (a h) e", e=HD + 1)
            nc.vector.memset(ones_view[:, :, HD:HD + 1], 1.0)
            return v_pad

        def qk_tile(qkT, xTb, rt):
            ps = mm_ps.tile([128, N], F32, tag="mm")
            for kc in range(KC):
                nc.tensor.matmul(
                    ps,
                    wqk[:, kc, rt * 128:(rt + 1) * 128],
                    xTb[:, kc, :],
                    start=(kc == 0),
                    stop=(kc == KC - 1),
                )
            nc.vector.tensor_copy(out=qkT[:, rt, :], in_=ps)

        def v_tile(v_pad, xTb, it, nh):
            ps = mm_ps.tile([128, C // 2], F32, tag="mm")
            for kc in range(KC):
                nc.tensor.matmul(
                    ps,
                    xTb[:, kc, it * 128:(it + 1) * 128],
                    wv[:, kc, nh * 384:(nh + 1) * 384],
                    start=(kc == 0),
                    stop=(kc == KC - 1),
                )
            dest = v_pad[:, it, :].rearrange("p (h e) -> p h e", e=HD + 1)
            nc.scalar.copy(
                out=dest[:, nh * 6:(nh + 1) * 6, 0:HD],
                in_=ps.rearrange("p (h e) -> p h e", e=HD),
            )

        def proj_tile(b, oT, it):
            outsb = pool_o.tile([128, C], F32, tag="outsb")
            for ct in range(2):
                ps = mm_ps.tile([128, C // 2], F32, tag="mm")
                for kc in range(KC):
                    nc.tensor.matmul(
                        ps,
                        oT[:, kc, it * 128:(it + 1) * 128],
                        wp[:, kc, ct * 384:(ct + 1) * 384],
                        start=(kc == 0),
                        stop=(kc == KC - 1),
                    )
                if PB_IS_ZERO:
                    nc.scalar.copy(out=outsb[:, ct * 384:(ct + 1) * 384], in_=ps)
                else:
                    nc.vector.tensor_add(
                        outsb[:, ct * 384:(ct + 1) * 384],
                        ps,
                        pb_bc[:, ct * 384:(ct + 1) * 384],
                    )
            nc.sync.dma_start(
                out=out_d[b * N + it * 128: b * N + (it + 1) * 128, :],
                in_=outsb,
            )

        def pair_scores(qkT, hp, jp):
            # head pair (hA even -> PE rows 0-63, hB odd -> rows 64-127):
            # their K=64 score matmuls execute concurrently on disjoint
            # row groups of the systolic array; I^T@biasT rides the same
            # PSUM accumulation; exp straight from PSUM, 1024 wide.
            rq, rk = hp, H // 2 + hp
            psA = s_ps.tile([128, 2, N], F32, tag="sTA")
            psB = s_ps.tile([128, 2, N], F32, tag="sTB")
            for jl in range(2):
                jt = jp * 2 + jl
                nc.tensor.matmul(
                    psA[:, jl, :],
                    qkT[0:64, rk, jt * 128:(jt + 1) * 128],
                    qkT[0:64, rq, :],
                    start=True,
                    stop=True,
                )
                nc.tensor.matmul(
                    psB[:, jl, :],
                    qkT[64:128, rk, jt * 128:(jt + 1) * 128],
                    qkT[64:128, rq, :],
                    start=True,
                    stop=True,
                )
            peA = pool_p.tile([128, 2, N], BF16, tag="peA")
            nc.scalar.activation(out=peA, in_=psA, func=Exp)
            peB = pool_p.tile([128, 2, N], BF16, tag="peB")
            nc.scalar.activation(out=peB, in_=psB, func=Exp)
            js = slice(jp * 2, jp * 2 + 2)
            ptA = pool_p.tile([128, 2, N], BF16, tag="pTA")
            nc.vector.tensor_mul(ptA, peA, biasT[:, 2 * hp, js, :])
            ptB = pool_p.tile([128, 2, N], BF16, tag="pTB")
            nc.vector.tensor_mul(ptB, peB, biasT[:, 2 * hp + 1, js, :])
            return ptA, ptB

        def head_out(oT, v_pad, h, pts):
            po = (h % 2) * 64
            rqo = h // 2
            pso = o_ps.tile([HD + 1, N], F32, tag="oT")
            for jt in range(NJT):
                vp = v_pad[:, jt, :].rearrange("p (h e) -> p h e", e=HD + 1)
                nc.tensor.matmul(
                    pso,
                    vp[:, h, :],
                    pts[jt // 2][:, jt % 2, :],
                    start=(jt == 0),
                    stop=(jt == NJT - 1),
                )
            nc.vector.tensor_copy(out=oT[po:po + 64, rqo, :], in_=pso[0:HD, :])
            smc = pool_r.tile([65, N], F32, tag="smc")
            nc.vector.tensor_copy(out=smc[HD:HD + 1, :], in_=pso[HD:HD + 1, :])
            rcd = dram_p.tile([1, N], F32, tag="rcd")
            nc.sync.dma_start(out=rcd, in_=smc[HD:HD + 1, :])
            return rcd

        def pair_norm(oT, hp, rcdA, rcdB):
            # one [128,N] tile: 1/sums(A) on partitions 0-63, 1/sums(B) on
            # 64-127 -> a single reciprocal + two in-place multiplies.
            rqo = hp
            rcf = pool_r.tile([128, N], F32, tag="rcf")
            nc.sync.dma_start(out=rcf[0:64, :], in_=rcdA[0:1, :].to_broadcast((64, N)))
            nc.sync.dma_start(out=rcf[64:128, :], in_=rcdB[0:1, :].to_broadcast((64, N)))
            nc.vector.reciprocal_approx_fast(out=rcf, in_=rcf)
            nc.gpsimd.tensor_mul(
                oT[0:64, rqo, :], oT[0:64, rqo, :], rcf[0:64, :]
            )
            nc.gpsimd.tensor_mul(
                oT[64:128, rqo, :], oT[64:128, rqo, :], rcf[64:128, :]
            )

        # ---- software-pipelined schedule ----
        # A persistent filler queue carries the next batch's qkv tiles and
        # the previous batch's projection tiles; pairs pop fillers between
        # their score and o matmuls, and dependencies are force-drained
        # just in time.  This keeps the PE dense through the last batch.
        work = []  # list of (key, fn); key=(kind, b, a, c)

        def drain(pred):
            i = 0
            while i < len(work):
                if pred(work[i][0]):
                    work.pop(i)[1]()
                else:
                    i += 1

        def pop_one():
            if work:
                work.pop(0)[1]()

        # startup: interleave wqk/x(0) chunk loads so the first matmul can
        # begin after one chunk of each.
        xT_cur = xt_pool.tile([128, KC, N], BF16, tag="xTb", name="xT0")
        for kc in range(KC):
            nc.sync.dma_start(
                out=wqk[:, kc, :], in_=wqk_d[kc * 128:(kc + 1) * 128, :]
            )
            nc.sync.dma_start(
                out=xT_cur[:, kc, :], in_=xT_d[kc * 128:(kc + 1) * 128, 0:N]
            )
        for kc in range(KC):
            nc.sync.dma_start(out=wv[:, kc, :], in_=wv_d[kc * 128:(kc + 1) * 128, :])
        qkT_cur = alloc_qkT()
        vp_cur = alloc_vpad()
        for rt in range(12):
            qk_tile(qkT_cur, xT_cur, rt)
        load_consts_late()
        for it in range(NIT):
            for nh in range(2):
                v_tile(vp_cur, xT_cur, it, nh)

        oT_prev, b_prev = None, None
        for b in range(BL):
            qkT, v_pad = qkT_cur, vp_cur
            fillers = []
            if b + 1 < BL:
                xT_nxt = load_x(b + 1)
                qkT_cur = alloc_qkT()
                vp_cur = alloc_vpad()
                fillers += [
                    (lambda rt=rt, q=qkT_cur, x=xT_nxt: qk_tile(q, x, rt))
                    for rt in range(12)
                ]
                fillers += [
                    (lambda it=it, nh=nh, v=vp_cur, x=xT_nxt:
                     v_tile(v, x, it, nh))
                    for it in range(NIT) for nh in range(2)
                ]
            if oT_prev is not None:
                fillers += [
                    (lambda it=it, ob=b_prev, ot=oT_prev: proj_tile(ob, ot, it))
                    for it in range(NIT)
                ]
            fi = 0
            oT = perb1.tile([128, KC, N], BF16, tag="oT")
            for hp in range(H // 2):
                pts01 = pair_scores(qkT, hp, 0)
                if fi < len(fillers):
                    fillers[fi](); fi += 1
                pts23 = pair_scores(qkT, hp, 1)
                if fi < len(fillers):
                    fillers[fi](); fi += 1
                ptsA = [pts01[0], pts23[0]]
                ptsB = [pts01[1], pts23[1]]
                rcdA = head_out(oT, v_pad, 2 * hp, ptsA)
                if fi < len(fillers):
                    fillers[fi](); fi += 1
                rcdB = head_out(oT, v_pad, 2 * hp + 1, ptsB)
                pair_norm(oT, hp, rcdA, rcdB)
            for f in fillers[fi:]:
                f()
            oT_prev, b_prev = oT, b
        for it in range(NIT):
            proj_tile(b_prev, oT_prev, it)
    nc.finalize()
    return nc


def kernel(x, qkv_w, proj_w, proj_b, bias_table, _trace=False, _tmpdir=None):
    x = np.asarray(x, dtype=np.float32)
    qkv_w = np.asarray(qkv_w, dtype=np.float32)
    proj_w = np.asarray(proj_w, dtype=np.float32)
    proj_b = np.asarray(proj_b, dtype=np.float32)
    bias_table = np.asarray(bias_table, dtype=np.float32)

    # host-side layout prep (weights + bias table expansion)
    wq_scaled = qkv_w.copy()
    wq_scaled[:C] *= HD ** (-0.5)
    wqk = np.ascontiguousarray(wq_scaled[: 2 * C].T).astype(ml_dtypes.bfloat16)
    wv = np.ascontiguousarray(qkv_w[2 * C:].T).astype(ml_dtypes.bfloat16)
    wp = np.ascontiguousarray(proj_w.T).astype(ml_dtypes.bfloat16)
    ii = np.arange(N)
    idx = ii[None, :] - ii[:, None] + (N - 1)                     # [j, i]
    biasT = np.ascontiguousarray(
        np.exp(bias_table[idx].transpose(2, 0, 1).reshape(H, NJT, 128, N))
    ).astype(ml_dtypes.bfloat16)
    pb = proj_b.reshape(1, C)

    global PB_IS_ZERO
    PB_IS_ZERO = not np.any(proj_b)
    nc = _build()
    in_maps = []
    for m in range(NCORES):
        xs = x[m * BL:(m + 1) * BL]                               # [8, 512, 768]
        xT = np.ascontiguousarray(xs.transpose(2, 0, 1).reshape(C, T)).astype(ml_dtypes.bfloat16)
        in_maps.append(
            {"xT": xT, "wqk": wqk, "wv": wv, "wp": wp, "biasT": biasT, "pb": pb}
        )
    res = run_bass_kernel_spmd(
        nc, in_maps, core_ids=list(range(NCORES)), trace=_trace, tmpdir=_tmpdir
    )
    out = np.concatenate(
        [res.results[m]["out"].reshape(BL, N, C) for m in range(NCORES)], axis=0
    )
    if _trace:
        return out, res
    return out


# revision 35
# speedup vs baseline: 1.1176x; 1.0574x over previous
"""Swin-style attention (B=64,N=512,C=768,H=12) on 8 TRN2 NeuronCores.

Strategy: pure data-parallel over batch (8 batches/core), no collectives.
Per core, one fused pipeline per batch:
  phase1: qkT = Wqk @ x^T (f32r matmuls), v = x @ Wv^T (natural layout,
          padded with a ones-column per head for fused softmax sums)
  attn:   per head h: sT[j,i] = kT^T@qT (K=64, f32r) -> +biasT (DVE)
          -> exp (ACT, bf16 out) -> oT~[d,i] & sums via [v|1] matmul (bf16)
          -> normalize rows by 1/sums (partition_broadcast + DVE mul)
  proj:   out = oT^T @ Wp^T + pb (bf16 matmul, f32r-grade accuracy not
          needed after softmax averaging)
Scale 1/8 is folded into the q-half of Wqk on the host; softmax runs
without max-subtraction (scores are O(1) by construction).
"""
import sys

sys.path.insert(0, "/opt/trn_rl_repo")
from contextlib import ExitStack

import ml_dtypes
import numpy as np

import concourse.bass as bass
import concourse.mybir as mybir
import concourse.tile as tile
from concourse import bacc
from concourse.bass_utils import run_bass_kernel_spmd
from concourse.masks import make_identity

F32 = mybir.dt.float32
F32R = mybir.dt.float32r
BF16 = mybir.dt.bfloat16

B, N, C, H, HD = 64, 512, 768, 12, 64
NCORES = 8
BL = B // NCORES          # batches per core
T = BL * N                # tokens per core
KC = C // 128             # 6 contraction chunks
NJT = N // 128            # 4 key-side tiles
NIT = N // 128            # 4 query/token tiles
VP = H * (HD + 1)         # 780: v padded with ones column per head
Exp = mybir.ActivationFunctionType.Exp


PB_IS_ZERO = True


def _build():
    nc = bacc.Bacc(target_bir_lowering=False)
    xT_d = nc.dram_tensor("xT", [C, T], BF16, kind="ExternalInput")
    wqk_d = nc.dram_tensor("wqk", [C, 2 * C], BF16, kind="ExternalInput")
    wv_d = nc.dram_tensor("wv", [C, C], BF16, kind="ExternalInput")
    wp_d = nc.dram_tensor("wp", [C, C], BF16, kind="ExternalInput")
    biasT_d = nc.dram_tensor("biasT", [H, NJT, 128, N], BF16, kind="ExternalInput")
    pb_d = nc.dram_tensor("pb", [1, C], F32, kind="ExternalInput")
    out_d = nc.dram_tensor("out", [T, C], F32, kind="ExternalOutput")

    with ExitStack() as ctx:
        tc = ctx.enter_context(tile.TileContext(nc))
        const = ctx.enter_context(tc.tile_pool(name="const", bufs=1))
        perb = ctx.enter_context(tc.tile_pool(name="perb", bufs=2))
        perb1 = ctx.enter_context(tc.tile_pool(name="perb1", bufs=2))
        xt_pool = ctx.enter_context(tc.tile_pool(name="xt", bufs=2))
        pool_p = ctx.enter_context(tc.tile_pool(name="pt", bufs=3))
        pool_r = ctx.enter_context(tc.tile_pool(name="rc", bufs=3))
        pool_o = ctx.enter_context(tc.tile_pool(name="osb", bufs=2))
        dram_p = ctx.enter_context(tc.tile_pool(name="dramp", bufs=2, space="DRAM"))
        mm_ps = ctx.enter_context(tc.tile_pool(name="mmps", bufs=3, space="PSUM"))
        s_ps = ctx.enter_context(tc.tile_pool(name="sps", bufs=1, space="PSUM"))
        o_ps = ctx.enter_context(tc.tile_pool(name="ops", bufs=1, space="PSUM"))

        # ---- constants ----
        wqk = const.tile([128, KC, 2 * C], BF16)
        wv = const.tile([128, KC, C], BF16)
        wp = const.tile([128, KC, C], BF16)
        biasT = const.tile([128, H, NJT, N], BF16)
        pb_bc = const.tile([128, C], F32)
        ident = const.tile([128, 128], BF16)
        make_identity(nc, ident)
        def load_consts_early():
            for kc in range(KC):
                nc.sync.dma_start(
                    out=wqk[:, kc, :], in_=wqk_d[kc * 128:(kc + 1) * 128, :]
                )
            for kc in range(KC):
                nc.sync.dma_start(
                    out=wv[:, kc, :], in_=wv_d[kc * 128:(kc + 1) * 128, :]
                )

        def load_consts_late():
            for kc in range(KC):
                nc.sync.dma_start(
                    out=wp[:, kc, :], in_=wp_d[kc * 128:(kc + 1) * 128, :]
                )
            for h in range(H):
                nc.sync.dma_start(
                    out=biasT[:, h, :, :],
                    in_=biasT_d[h, :, :, :].rearrange("a p b -> p a b"),
                )
            nc.sync.dma_start(out=pb_bc, in_=pb_d[0:1, :].to_broadcast((128, C)))

        def load_x(b):
            xTb = xt_pool.tile([128, KC, N], BF16, tag="xTb")
            for kc in range(KC):
                nc.sync.dma_start(
                    out=xTb[:, kc, :],
                    in_=xT_d[kc * 128:(kc + 1) * 128, b * N:(b + 1) * N],
                )
            return xTb

        def alloc_qkT():
            return perb.tile([128, 2 * H // 2, N], BF16, tag="qkT", name="qkT")

        def alloc_vpad():
            v_pad = perb.tile([128, NIT, VP], BF16, tag="v_pad")
            ones_view = v_pad.rearrange("p a (h e) -> p (a h) e", e=HD + 1)
            nc.vector.memset(ones_view[:, :, HD:HD + 1], 1.0)
            return v_pad

        def qk_tile(qkT, xTb, rt):
            ps = mm_ps.tile([128, N], F32, tag="mm")
            for kc in range(KC):
                nc.tensor.matmul(
                    ps,
                    wqk[:, kc, rt * 128:(rt + 1) * 128],
                    xTb[:, kc, :],
                    start=(kc == 0),
                    stop=(kc == KC - 1),
                )
            nc.vector.tensor_copy(out=qkT[:, rt, :], in_=ps)

        def v_tile(v_pad, xTb, it, nh):
            ps = mm_ps.tile([128, C // 2], F32, tag="mm")
            for kc in range(KC):
                nc.tensor.matmul(
                    ps,
                    xTb[:, kc, it * 128:(it + 1) * 128],
                    wv[:, kc, nh * 384:(nh + 1) * 384],
                    start=(kc == 0),
                    stop=(kc == KC - 1),
                )
            dest = v_pad[:, it, :].rearrange("p (h e) -> p h e", e=HD + 1)
            nc.scalar.copy(
                out=dest[:, nh * 6:(nh + 1) * 6, 0:HD],
                in_=ps.rearrange("p (h e) -> p h e", e=HD),
            )

        def proj_tile(b, oT, it):
            outsb = pool_o.tile([128, C], F32, tag="outsb")
            for ct in range(2):
                ps = mm_ps.tile([128, C // 2], F32, tag="mm")
                for kc in range(KC):
                    nc.tensor.matmul(
                        ps,
                        oT[:, kc, it * 128:(it + 1) * 128],
                        wp[:, kc, ct * 384:(ct + 1) * 384],
                        start=(kc == 0),
                        stop=(kc == KC - 1),
                    )
                if PB_IS_ZERO:
                    nc.scalar.copy(out=outsb[:, ct * 384:(ct + 1) * 384], in_=ps)
                else:
                    nc.vector.tensor_add(
                        outsb[:, ct * 384:(ct + 1) * 384],
                        ps,
                        pb_bc[:, ct * 384:(ct + 1) * 384],
                    )
            nc.sync.dma_start(
                out=out_d[b * N + it * 128: b * N + (it + 1) * 128, :],
                in_=outsb,
            )

        def pair_scores(qkT, hp, jp):
            # head pair (hA even -> PE rows 0-63, hB odd -> rows 64-127):
            # their K=64 score matmuls execute concurrently on disjoint
            # row groups of the systolic array; I^T@biasT rides the same
            # PSUM accumulation; exp straight from PSUM, 1024 wide.
            rq, rk = hp, H // 2 + hp
            psA = s_ps.tile([128, 2, N], F32, tag="sTA")
            psB = s_ps.tile([128, 2, N], F32, tag="sTB")
            for jl in range(2):
                jt = jp * 2 + jl
                nc.tensor.matmul(
                    psA[:, jl, :],
                    qkT[0:64, rk, jt * 128:(jt + 1) * 128],
                    qkT[0:64, rq, :],
                    start=True,
                    stop=True,
                )
                nc.tensor.matmul(
                    psB[:, jl, :],
                    qkT[64:128, rk, jt * 128:(jt + 1) * 128],
                    qkT[64:128, rq, :],
                    start=True,
                    stop=True,
                )
            peA = pool_p.tile([128, 2, N], BF16, tag="peA")
            nc.scalar.activation(out=peA, in_=psA, func=Exp)
            peB = pool_p.tile([128, 2, N], BF16, tag="peB")
            nc.scalar.activation(out=peB, in_=psB, func=Exp)
            js = slice(jp * 2, jp * 2 + 2)
            ptA = pool_p.tile([128, 2, N], BF16, tag="pTA")
            nc.vector.tensor_mul(ptA, peA, biasT[:, 2 * hp, js, :])
            ptB = pool_p.tile([128, 2, N], BF16, tag="pTB")
            nc.vector.tensor_mul(ptB, peB, biasT[:, 2 * hp + 1, js, :])
            return ptA, ptB

        def head_out(oT, v_pad, h, pts):
            po = (h % 2) * 64
            rqo = h // 2
            pso = o_ps.tile([HD + 1, N], F32, tag="oT")
            for jt in range(NJT):
                vp = v_pad[:, jt, :].rearrange("p (h e) -> p h e", e=HD + 1)
                nc.tensor.matmul(
                    pso,
                    vp[:, h, :],
                    pts[jt // 2][:, jt % 2, :],
                    start=(jt == 0),
                    stop=(jt == NJT - 1),
                )
            nc.vector.tensor_copy(out=oT[po:po + 64, rqo, :], in_=pso[0:HD, :])
            smc = pool_r.tile([65, N], F32, tag="smc")
            nc.vector.tensor_copy(out=smc[HD:HD + 1, :], in_=pso[HD:HD + 1, :])
            rcd = dram_p.tile([1, N], F32, tag="rcd")
            nc.sync.dma_start(out=rcd, in_=smc[HD:HD + 1, :])
            return rcd

        def pair_norm(oT, hp, rcdA, rcdB):
            # one [128,N] tile: 1/sums(A) on partitions 0-63, 1/sums(B) on
            # 64-127 -> a single reciprocal + two in-place multiplies.
            rqo = hp
            rcf = pool_r.tile([128, N], F32, tag="rcf")
            nc.sync.dma_start(out=rcf[0:64, :], in_=rcdA[0:1, :].to_broadcast((64, N)))
            nc.sync.dma_start(out=rcf[64:128, :], in_=rcdB[0:1, :].to_broadcast((64, N)))
            nc.vector.reciprocal_approx_fast(out=rcf, in_=rcf)
            nc.gpsimd.tensor_mul(
                oT[0:64, rqo, :], oT[0:64, rqo, :], rcf[0:64, :]
            )
            nc.gpsimd.tensor_mul(
                oT[64:128, rqo, :], oT[64:128, rqo, :], rcf[64:128, :]
            )

        # ---- software-pipelined schedule ----
        # A persistent filler queue carries the next batch's qkv tiles and
        # the previous batch's projection tiles; pairs pop fillers between
        # their score and o matmuls, and dependencies are force-drained
        # just in time.  This keeps the PE dense through the last batch.
        work = []  # list of (key, fn); key=(kind, b, a, c)

        def drain(pred):
            i = 0
            while i < len(work):
                if pred(work[i][0]):
                    work.pop(i)[1]()
                else:
                    i += 1

        def pop_one():
            if work:
                work.pop(0)[1]()

        # startup: interleave wqk/x(0) chunk loads so the first matmul can
        # begin after one chunk of each.
        xT_cur = xt_pool.tile([128, KC, N], BF16, tag="xTb", name="xT0")
        for kc in range(KC):
            nc.sync.dma_start(
                out=wqk[:, kc, :], in_=wqk_d[kc * 128:(kc + 1) * 128, :]
            )
            nc.sync.dma_start(
                out=xT_cur[:, kc, :], in_=xT_d[kc * 128:(kc + 1) * 128, 0:N]
            )
        for kc in range(KC):
            nc.sync.dma_start(out=wv[:, kc, :], in_=wv_d[kc * 128:(kc + 1) * 128, :])
        qkT_cur = alloc_qkT()
        vp_cur = alloc_vpad()
        for rt in range(12):
            qk_tile(qkT_cur, xT_cur, rt)
        load_consts_late()
        for it in range(NIT):
            for nh in range(2):
                v_tile(vp_cur, xT_cur, it, nh)

        oT_prev, b_prev = None, None
        for b in range(BL):
            qkT, v_pad = qkT_cur, vp_cur
            fillers = []
            if b + 1 < BL:
                xT_nxt = load_x(b + 1)
                qkT_cur = alloc_qkT()
                vp_cur = alloc_vpad()
                fillers += [
                    (lambda rt=rt, q=qkT_cur, x=xT_nxt: qk_tile(q, x, rt))
                    for rt in range(12)
                ]
                fillers += [
                    (lambda it=it, nh=nh, v=vp_cur, x=xT_nxt:
                     v_tile(v, x, it, nh))
                    for it in range(NIT) for nh in range(2)
                ]
            if oT_prev is not None:
                fillers += [
                    (lambda it=it, ob=b_prev, ot=oT_prev: proj_tile(ob, ot, it))
                    for it in range(NIT)
                ]
            fi = 0
            oT = perb1.tile([128, KC, N], BF16, tag="oT")
            for hp in range(H // 2):
                pts01 = pair_scores(qkT, hp, 0)
                if fi < len(fillers):
                    fillers[fi](); fi += 1
                pts23 = pair_scores(qkT, hp, 1)
                if fi < len(fillers):
                    fillers[fi](); fi += 1
                ptsA = [pts01[0], pts23[0]]
                ptsB = [pts01[1], pts23[1]]
                rcdA = head_out(oT, v_pad, 2 * hp, ptsA)
                if fi < len(fillers):
                    fillers[fi](); fi += 1
                rcdB = head_out(oT, v_pad, 2 * hp + 1, ptsB)
                if fi < len(fillers):
                    fillers[fi](); fi += 1
                pair_norm(oT, hp, rcdA, rcdB)
            for f in fillers[fi:]:
                f()
            oT_prev, b_prev = oT, b
        for it in range(NIT):
            proj_tile(b_prev, oT_prev, it)
    nc.finalize()
    return nc


def kernel(x, qkv_w, proj_w, proj_b, bias_table, _trace=False, _tmpdir=None):
    x = np.asarray(x, dtype=np.float32)
    qkv_w = np.asarray(qkv_w, dtype=np.float32)
    proj_w = np.asarray(proj_w, dtype=np.float32)
    proj_b = np.asarray(proj_b, dtype=np.float32)
    bias_table = np.asarray(bias_table, dtype=np.float32)

    # host-side layout prep (weights + bias table expansion)
    wq_scaled = qkv_w.copy()
    wq_scaled[:C] *= HD ** (-0.5)
    wqk = np.ascontiguousarray(wq_scaled[: 2 * C].T).astype(ml_dtypes.bfloat16)
    wv = np.ascontiguousarray(qkv_w[2 * C:].T).astype(ml_dtypes.bfloat16)
    wp = np.ascontiguousarray(proj_w.T).astype(ml_dtypes.bfloat16)
    ii = np.arange(N)
    idx = ii[None, :] - ii[:, None] + (N - 1)                     # [j, i]
    biasT = np.ascontiguousarray(
        np.exp(bias_table[idx].transpose(2, 0, 1).reshape(H, NJT, 128, N))
    ).astype(ml_dtypes.bfloat16)
    pb = proj_b.reshape(1, C)

    global PB_IS_ZERO
    PB_IS_ZERO = not np.any(proj_b)
    nc = _build()
    in_maps = []
    for m in range(NCORES):
        xs = x[m * BL:(m + 1) * BL]                               # [8, 512, 768]
        xT = np.ascontiguousarray(xs.transpose(2, 0, 1).reshape(C, T)).astype(ml_dtypes.bfloat16)
        in_maps.append(
            {"xT": xT, "wqk": wqk, "wv": wv, "wp": wp, "biasT": biasT, "pb": pb}
        )
    res = run_bass_kernel_spmd(
        nc, in_maps, core_ids=list(range(NCORES)), trace=_trace, tmpdir=_tmpdir
    )
    out = np.concatenate(
        [res.results[m]["out"].reshape(BL, N, C) for m in range(NCORES)], axis=0
    )
    if _trace:
        return out, res
    return out


# revision 38
# speedup vs baseline: 1.1340x; 1.0147x over previous
"""Swin-style attention (B=64,N=512,C=768,H=12) on 8 TRN2 NeuronCores.

Strategy: pure data-parallel over batch (8 batches/core), no collectives.
Per core, one fused pipeline per batch:
  phase1: qkT = Wqk @ x^T (f32r matmuls), v = x @ Wv^T (natural layout,
          padded with a ones-column per head for fused softmax sums)
  attn:   per head h: sT[j,i] = kT^T@qT (K=64, f32r) -> +biasT (DVE)
          -> exp (ACT, bf16 out) -> oT~[d,i] & sums via [v|1] matmul (bf16)
          -> normalize rows by 1/sums (partition_broadcast + DVE mul)
  proj:   out = oT^T @ Wp^T + pb (bf16 matmul, f32r-grade accuracy not
          needed after softmax averaging)
Scale 1/8 is folded into the q-half of Wqk on the host; softmax runs
without max-subtraction (scores are O(1) by construction).
"""
import sys

sys.path.insert(0, "/opt/trn_rl_repo")
from contextlib import ExitStack

import ml_dtypes
import numpy as np

import concourse.bass as bass
import concourse.mybir as mybir
import concourse.tile as tile
from concourse import bacc
from concourse.bass_utils import run_bass_kernel_spmd
from concourse.masks import make_identity

F32 = mybir.dt.float32
F32R = mybir.dt.float32r
BF16 = mybir.dt.bfloat16

B, N, C, H, HD = 64, 512, 768, 12, 64
NCORES = 8
BL = B // NCORES          # batches per core
T = BL * N                # tokens per core
KC = C // 128             # 6 contraction chunks
NJT = N // 128            # 4 key-side tiles
NIT = N // 128            # 4 query/token tiles
VP = H * (HD + 1)         # 780: v padded with ones column per head
Exp = mybir.ActivationFunctionType.Exp


PB_IS_ZERO = True


def _build():
    nc = bacc.Bacc(target_bir_lowering=False)
    xT_d = nc.dram_tensor("xT", [C, T], BF16, kind="ExternalInput")
    wqk_d = nc.dram_tensor("wqk", [C, 2 * C], BF16, kind="ExternalInput")
    wv_d = nc.dram_tensor("wv", [C, C], BF16, kind="ExternalInput")
    wp_d = nc.dram_tensor("wp", [C, C], BF16, kind="ExternalInput")
    biasT_d = nc.dram_tensor("biasT", [H, NJT, 128, N], BF16, kind="ExternalInput")
    pb_d = nc.dram_tensor("pb", [1, C], F32, kind="ExternalInput")
    out_d = nc.dram_tensor("out", [T, C], F32, kind="ExternalOutput")

    with ExitStack() as ctx:
        tc = ctx.enter_context(tile.TileContext(nc))
        const = ctx.enter_context(tc.tile_pool(name="const", bufs=1))
        perb = ctx.enter_context(tc.tile_pool(name="perb", bufs=2))
        perb1 = ctx.enter_context(tc.tile_pool(name="perb1", bufs=2))
        xt_pool = ctx.enter_context(tc.tile_pool(name="xt", bufs=2))
        pool_p = ctx.enter_context(tc.tile_pool(name="pt", bufs=3))
        pool_r = ctx.enter_context(tc.tile_pool(name="rc", bufs=3))
        pool_o = ctx.enter_context(tc.tile_pool(name="osb", bufs=2))
        dram_p = ctx.enter_context(tc.tile_pool(name="dramp", bufs=2, space="DRAM"))
        mm_ps = ctx.enter_context(tc.tile_pool(name="mmps", bufs=3, space="PSUM"))
        s_ps = ctx.enter_context(tc.tile_pool(name="sps", bufs=1, space="PSUM"))
        o_ps = ctx.enter_context(tc.tile_pool(name="ops", bufs=1, space="PSUM"))

        # ---- constants ----
        wqk = const.tile([128, KC, 2 * C], BF16)
        wv = const.tile([128, KC, C], BF16)
        wp = const.tile([128, KC, C], BF16)
        biasT = const.tile([128, H, NJT, N], BF16)
        pb_bc = const.tile([128, C], F32)
        ident = const.tile([128, 128], BF16)
        make_identity(nc, ident)
        def load_consts_early():
            for kc in range(KC):
                nc.sync.dma_start(
                    out=wqk[:, kc, :], in_=wqk_d[kc * 128:(kc + 1) * 128, :]
                )
            for kc in range(KC):
                nc.sync.dma_start(
                    out=wv[:, kc, :], in_=wv_d[kc * 128:(kc + 1) * 128, :]
                )

        def load_consts_late():
            for kc in range(KC):
                nc.sync.dma_start(
                    out=wp[:, kc, :], in_=wp_d[kc * 128:(kc + 1) * 128, :]
                )
            for h in range(H):
                nc.sync.dma_start(
                    out=biasT[:, h, :, :],
                    in_=biasT_d[h, :, :, :].rearrange("a p b -> p a b"),
                )
            nc.sync.dma_start(out=pb_bc, in_=pb_d[0:1, :].to_broadcast((128, C)))

        def load_x(b):
            xTb = xt_pool.tile([128, KC, N], BF16, tag="xTb")
            for kc in range(KC):
                nc.sync.dma_start(
                    out=xTb[:, kc, :],
                    in_=xT_d[kc * 128:(kc + 1) * 128, b * N:(b + 1) * N],
                )
            return xTb

        def alloc_qkT():
            return perb.tile([128, 2 * H // 2, N], BF16, tag="qkT", name="qkT")

        def alloc_vpad():
            v_pad = perb.tile([128, NIT, VP], BF16, tag="v_pad")
            ones_view = v_pad.rearrange("p a (h e) -> p (a h) e", e=HD + 1)
            nc.vector.memset(ones_view[:, :, HD:HD + 1], 1.0)
            return v_pad

        def qk_tile(qkT, xTb, rt):
            ps = mm_ps.tile([128, N], F32, tag="mm")
            for kc in range(KC):
                nc.tensor.matmul(
                    ps,
                    wqk[:, kc, rt * 128:(rt + 1) * 128],
                    xTb[:, kc, :],
                    start=(kc == 0),
                    stop=(kc == KC - 1),
                )
            nc.vector.tensor_copy(out=qkT[:, rt, :], in_=ps)

        def v_tile(v_pad, xTb, it, nh):
            ps = mm_ps.tile([128, C // 2], F32, tag="mm")
            for kc in range(KC):
                nc.tensor.matmul(
                    ps,
                    xTb[:, kc, it * 128:(it + 1) * 128],
                    wv[:, kc, nh * 384:(nh + 1) * 384],
                    start=(kc == 0),
                    stop=(kc == KC - 1),
                )
            dest = v_pad[:, it, :].rearrange("p (h e) -> p h e", e=HD + 1)
            nc.scalar.copy(
                out=dest[:, nh * 6:(nh + 1) * 6, 0:HD],
                in_=ps.rearrange("p (h e) -> p h e", e=HD),
            )

        def proj_tile(b, oT, it):
            outsb = pool_o.tile([128, C], F32, tag="outsb")
            for ct in range(2):
                ps = mm_ps.tile([128, C // 2], F32, tag="mm")
                for kc in range(KC):
                    nc.tensor.matmul(
                        ps,
                        oT[:, kc, it * 128:(it + 1) * 128],
                        wp[:, kc, ct * 384:(ct + 1) * 384],
                        start=(kc == 0),
                        stop=(kc == KC - 1),
                    )
                if PB_IS_ZERO:
                    nc.scalar.copy(out=outsb[:, ct * 384:(ct + 1) * 384], in_=ps)
                else:
                    nc.vector.tensor_add(
                        outsb[:, ct * 384:(ct + 1) * 384],
                        ps,
                        pb_bc[:, ct * 384:(ct + 1) * 384],
                    )
            nc.sync.dma_start(
                out=out_d[b * N + it * 128: b * N + (it + 1) * 128, :],
                in_=outsb,
            )

        def pair_scores(qkT, hp, jp):
            # head pair (hA even -> PE rows 0-63, hB odd -> rows 64-127):
            # their K=64 score matmuls execute concurrently on disjoint
            # row groups of the systolic array; I^T@biasT rides the same
            # PSUM accumulation; exp straight from PSUM, 1024 wide.
            rq, rk = hp, H // 2 + hp
            psA = s_ps.tile([128, 2, N], F32, tag="sTA")
            psB = s_ps.tile([128, 2, N], F32, tag="sTB")
            for jl in range(2):
                jt = jp * 2 + jl
                nc.tensor.matmul(
                    psA[:, jl, :],
                    qkT[0:64, rk, jt * 128:(jt + 1) * 128],
                    qkT[0:64, rq, :],
                    start=True,
                    stop=True,
                )
                nc.tensor.matmul(
                    psB[:, jl, :],
                    qkT[64:128, rk, jt * 128:(jt + 1) * 128],
                    qkT[64:128, rq, :],
                    start=True,
                    stop=True,
                )
            peA = pool_p.tile([128, 2, N], BF16, tag="peA")
            nc.scalar.activation(out=peA, in_=psA, func=Exp)
            peB = pool_p.tile([128, 2, N], BF16, tag="peB")
            nc.scalar.activation(out=peB, in_=psB, func=Exp)
            js = slice(jp * 2, jp * 2 + 2)
            ptA = pool_p.tile([128, 2, N], BF16, tag="pTA")
            nc.vector.tensor_mul(ptA, peA, biasT[:, 2 * hp, js, :])
            ptB = pool_p.tile([128, 2, N], BF16, tag="pTB")
            nc.vector.tensor_mul(ptB, peB, biasT[:, 2 * hp + 1, js, :])
            return ptA, ptB

        def head_out(oT, v_pad, h, pts):
            po = (h % 2) * 64
            rqo = h // 2
            pso = o_ps.tile([HD + 1, N], F32, tag="oT")
            for jt in range(NJT):
                vp = v_pad[:, jt, :].rearrange("p (h e) -> p h e", e=HD + 1)
                nc.tensor.matmul(
                    pso,
                    vp[:, h, :],
                    pts[jt // 2][:, jt % 2, :],
                    start=(jt == 0),
                    stop=(jt == NJT - 1),
                )
            nc.vector.tensor_copy(out=oT[po:po + 64, rqo, :], in_=pso[0:HD, :])
            smc = pool_r.tile([65, N], F32, tag="smc")
            nc.vector.tensor_copy(out=smc[HD:HD + 1, :], in_=pso[HD:HD + 1, :])
            rcd = dram_p.tile([1, N], F32, tag="rcd")
            nc.sync.dma_start(out=rcd, in_=smc[HD:HD + 1, :])
            return rcd

        def pair_norm(oT, hp, rcdA, rcdB):
            # one [128,N] tile: 1/sums(A) on partitions 0-63, 1/sums(B) on
            # 64-127 -> a single reciprocal + two in-place multiplies.
            rqo = hp
            rcf = pool_r.tile([128, N], F32, tag="rcf")
            nc.sync.dma_start(out=rcf[0:64, :], in_=rcdA[0:1, :].to_broadcast((64, N)))
            nc.sync.dma_start(out=rcf[64:128, :], in_=rcdB[0:1, :].to_broadcast((64, N)))
            nc.vector.reciprocal_approx_fast(out=rcf, in_=rcf)
            nc.gpsimd.tensor_mul(
                oT[0:64, rqo, :], oT[0:64, rqo, :], rcf[0:64, :]
            )
            nc.gpsimd.tensor_mul(
                oT[64:128, rqo, :], oT[64:128, rqo, :], rcf[64:128, :]
            )

        # ---- software-pipelined schedule ----
        # A persistent filler queue carries the next batch's qkv tiles and
        # the previous batch's projection tiles; pairs pop fillers between
        # their score and o matmuls, and dependencies are force-drained
        # just in time.  This keeps the PE dense through the last batch.
        work = []  # list of (key, fn); key=(kind, b, a, c)

        def drain(pred):
            i = 0
            while i < len(work):
                if pred(work[i][0]):
                    work.pop(i)[1]()
                else:
                    i += 1

        def pop_one():
            if work:
                work.pop(0)[1]()

        # startup: interleave wqk/x(0) chunk loads so the first matmul can
        # begin after one chunk of each.
        xT_cur = xt_pool.tile([128, KC, N], BF16, tag="xTb", name="xT0")
        for kc in range(KC):
            nc.sync.dma_start(
                out=wqk[:, kc, :], in_=wqk_d[kc * 128:(kc + 1) * 128, :]
            )
            nc.sync.dma_start(
                out=xT_cur[:, kc, :], in_=xT_d[kc * 128:(kc + 1) * 128, 0:N]
            )
        for kc in range(KC):
            nc.sync.dma_start(out=wv[:, kc, :], in_=wv_d[kc * 128:(kc + 1) * 128, :])
        qkT_cur = alloc_qkT()
        vp_cur = alloc_vpad()
        for rt in range(12):
            qk_tile(qkT_cur, xT_cur, rt)
        load_consts_late()
        for it in range(NIT):
            for nh in range(2):
                v_tile(vp_cur, xT_cur, it, nh)

        oT_prev, b_prev = None, None
        deferred = []
        for b in range(BL):
            qkT, v_pad = qkT_cur, vp_cur
            fillers = list(deferred)
            deferred = []
            if b + 1 < BL:
                xT_nxt = load_x(b + 1)
                qkT_cur = alloc_qkT()
                vp_cur = alloc_vpad()
                if b + 1 == BL - 1:
                    # the last batch has no successor to fill its pair gaps:
                    # run only the tiles its first pairs need now, defer the
                    # rest (in dependency-safe order) as its own fillers.
                    early_rt = [0, 6, 1, 7]
                    late_rt = [2, 8, 3, 9, 4, 10, 5, 11]
                    fillers += [
                        (lambda rt=rt, q=qkT_cur, x=xT_nxt: qk_tile(q, x, rt))
                        for rt in early_rt
                    ]
                    fillers += [
                        (lambda it=it, v=vp_cur, x=xT_nxt: v_tile(v, x, it, 0))
                        for it in range(NIT)
                    ]
                    deferred += [
                        (lambda rt=rt, q=qkT_cur, x=xT_nxt: qk_tile(q, x, rt))
                        for rt in late_rt[:6]
                    ]
                    deferred += [
                        (lambda it=it, v=vp_cur, x=xT_nxt: v_tile(v, x, it, 1))
                        for it in range(2)
                    ]
                    deferred += [
                        (lambda rt=rt, q=qkT_cur, x=xT_nxt: qk_tile(q, x, rt))
                        for rt in late_rt[6:]
                    ]
                    deferred += [
                        (lambda it=it, v=vp_cur, x=xT_nxt: v_tile(v, x, it, 1))
                        for it in range(2, NIT)
                    ]
                else:
                    fillers += [
                        (lambda rt=rt, q=qkT_cur, x=xT_nxt: qk_tile(q, x, rt))
                        for rt in range(12)
                    ]
                    fillers += [
                        (lambda it=it, nh=nh, v=vp_cur, x=xT_nxt:
                         v_tile(v, x, it, nh))
                        for it in range(NIT) for nh in range(2)
                    ]
            if oT_prev is not None:
                fillers += [
                    (lambda it=it, ob=b_prev, ot=oT_prev: proj_tile(ob, ot, it))
                    for it in range(NIT)
                ]
            # spread fillers evenly over the 24 pair slots; the last batch's
            # fillers carry intra-batch dependencies, so keep their order and
            # pack them densely from the front instead.
            slots = [None] * 24
            nf = len(fillers)
            if nf:
                if b == BL - 1:
                    for i, f in enumerate(fillers[:24]):
                        slots[i] = f
                else:
                    for i, f in enumerate(fillers[:24]):
                        slots[(i * 24) // min(nf, 24)] = f
            extra = fillers[24:]

            def use(si):
                if slots[si] is not None:
                    slots[si]()

            oT = perb1.tile([128, KC, N], BF16, tag="oT")
            for hp in range(H // 2):
                pts01 = pair_scores(qkT, hp, 0)
                use(hp * 4)
                pts23 = pair_scores(qkT, hp, 1)
                use(hp * 4 + 1)
                ptsA = [pts01[0], pts23[0]]
                ptsB = [pts01[1], pts23[1]]
                rcdA = head_out(oT, v_pad, 2 * hp, ptsA)
                use(hp * 4 + 2)
                rcdB = head_out(oT, v_pad, 2 * hp + 1, ptsB)
                use(hp * 4 + 3)
                pair_norm(oT, hp, rcdA, rcdB)
            for f in extra:
                f()
            oT_prev, b_prev = oT, b
        for it in range(NIT):
            proj_tile(b_prev, oT_prev, it)
    nc.finalize()
    return nc


def kernel(x, qkv_w, proj_w, proj_b, bias_table, _trace=False, _tmpdir=None):
    x = np.asarray(x, dtype=np.float32)
    qkv_w = np.asarray(qkv_w, dtype=np.float32)
    proj_w = np.asarray(proj_w, dtype=np.float32)
    proj_b = np.asarray(proj_b, dtype=np.float32)
    bias_table = np.asarray(bias_table, dtype=np.float32)

    # host-side layout prep (weights + bias table expansion)
    wq_scaled = qkv_w.copy()
    wq_scaled[:C] *= HD ** (-0.5)
    wqk = np.ascontiguousarray(wq_scaled[: 2 * C].T).astype(ml_dtypes.bfloat16)
    wv = np.ascontiguousarray(qkv_w[2 * C:].T).astype(ml_dtypes.bfloat16)
    wp = np.ascontiguousarray(proj_w.T).astype(ml_dtypes.bfloat16)
    ii = np.arange(N)
    idx = ii[None, :] - ii[:, None] + (N - 1)                     # [j, i]
    biasT = np.ascontiguousarray(
        np.exp(bias_table[idx].transpose(2, 0, 1).reshape(H, NJT, 128, N))
    ).astype(ml_dtypes.bfloat16)
    pb = proj_b.reshape(1, C)

    global PB_IS_ZERO
    PB_IS_ZERO = not np.any(proj_b)
    nc = _build()
    in_maps = []
    for m in range(NCORES):
        xs = x[m * BL:(m + 1) * BL]                               # [8, 512, 768]
        xT = np.ascontiguousarray(xs.transpose(2, 0, 1).reshape(C, T)).astype(ml_dtypes.bfloat16)
        in_maps.append(
            {"xT": xT, "wqk": wqk, "wv": wv, "wp": wp, "biasT": biasT, "pb": pb}
        )
    res = run_bass_kernel_spmd(
        nc, in_maps, core_ids=list(range(NCORES)), trace=_trace, tmpdir=_tmpdir
    )
    out = np.concatenate(
        [res.results[m]["out"].reshape(BL, N, C) for m in range(NCORES)], axis=0
    )
    if _trace:
        return out, res
    return out


# revision 41
# speedup vs baseline: 1.1576x; 1.0208x over previous
"""Swin-style attention (B=64,N=512,C=768,H=12) on 8 TRN2 NeuronCores.

Strategy: pure data-parallel over batch (8 batches/core), no collectives.
Per core, one fused pipeline per batch:
  phase1: qkT = Wqk @ x^T (f32r matmuls), v = x @ Wv^T (natural layout,
          padded with a ones-column per head for fused softmax sums)
  attn:   per head h: sT[j,i] = kT^T@qT (K=64, f32r) -> +biasT (DVE)
          -> exp (ACT, bf16 out) -> oT~[d,i] & sums via [v|1] matmul (bf16)
          -> normalize rows by 1/sums (partition_broadcast + DVE mul)
  proj:   out = oT^T @ Wp^T + pb (bf16 matmul, f32r-grade accuracy not
          needed after softmax averaging)
Scale 1/8 is folded into the q-half of Wqk on the host; softmax runs
without max-subtraction (scores are O(1) by construction).
"""
import sys

sys.path.insert(0, "/opt/trn_rl_repo")
from contextlib import ExitStack

import ml_dtypes
import numpy as np

import concourse.bass as bass
import concourse.mybir as mybir
import concourse.tile as tile
from concourse import bacc
from concourse.bass_utils import run_bass_kernel_spmd
from concourse.masks import make_identity

F32 = mybir.dt.float32
F32R = mybir.dt.float32r
BF16 = mybir.dt.bfloat16

B, N, C, H, HD = 64, 512, 768, 12, 64
NCORES = 8
BL = B // NCORES          # batches per core
T = BL * N                # tokens per core
KC = C // 128             # 6 contraction chunks
NJT = N // 128            # 4 key-side tiles
NIT = N // 128            # 4 query/token tiles
VP = H * (HD + 1)         # 780: v padded with ones column per head
Exp = mybir.ActivationFunctionType.Exp


PB_IS_ZERO = True


def _build():
    nc = bacc.Bacc(target_bir_lowering=False)
    xT_d = nc.dram_tensor("xT", [C, T], BF16, kind="ExternalInput")
    wqk_d = nc.dram_tensor("wqk", [C, 2 * C], BF16, kind="ExternalInput")
    wv_d = nc.dram_tensor("wv", [C, C], BF16, kind="ExternalInput")
    wp_d = nc.dram_tensor("wp", [C, C], BF16, kind="ExternalInput")
    biasT_d = nc.dram_tensor("biasT", [H, NJT, 128, N], BF16, kind="ExternalInput")
    pb_d = nc.dram_tensor("pb", [1, C], F32, kind="ExternalInput")
    out_d = nc.dram_tensor("out", [T, C], F32, kind="ExternalOutput")

    with ExitStack() as ctx:
        tc = ctx.enter_context(tile.TileContext(nc))
        const = ctx.enter_context(tc.tile_pool(name="const", bufs=1))
        perb = ctx.enter_context(tc.tile_pool(name="perb", bufs=2))
        perb1 = ctx.enter_context(tc.tile_pool(name="perb1", bufs=2))
        xt_pool = ctx.enter_context(tc.tile_pool(name="xt", bufs=2))
        pool_p = ctx.enter_context(tc.tile_pool(name="pt", bufs=2))
        pool_r = ctx.enter_context(tc.tile_pool(name="rc", bufs=3))
        pool_o = ctx.enter_context(tc.tile_pool(name="osb", bufs=2))
        dram_p = ctx.enter_context(tc.tile_pool(name="dramp", bufs=2, space="DRAM"))
        mm_ps = ctx.enter_context(tc.tile_pool(name="mmps", bufs=3, space="PSUM"))
        s_ps = ctx.enter_context(tc.tile_pool(name="sps", bufs=1, space="PSUM"))
        o_ps = ctx.enter_context(tc.tile_pool(name="ops", bufs=1, space="PSUM"))

        # ---- constants ----
        wqk = const.tile([128, KC, 2 * C], BF16)
        wv = const.tile([128, KC, C], BF16)
        wp = const.tile([128, KC, C], BF16)
        biasT = const.tile([128, H, NJT, N], BF16)
        pb_bc = const.tile([128, C], F32)
        ident = const.tile([128, 128], BF16)
        make_identity(nc, ident)
        def load_consts_early():
            for kc in range(KC):
                nc.sync.dma_start(
                    out=wqk[:, kc, :], in_=wqk_d[kc * 128:(kc + 1) * 128, :]
                )
            for kc in range(KC):
                nc.sync.dma_start(
                    out=wv[:, kc, :], in_=wv_d[kc * 128:(kc + 1) * 128, :]
                )

        def load_consts_late():
            for kc in range(KC):
                nc.sync.dma_start(
                    out=wp[:, kc, :], in_=wp_d[kc * 128:(kc + 1) * 128, :]
                )
            for h in range(H):
                nc.sync.dma_start(
                    out=biasT[:, h, :, :],
                    in_=biasT_d[h, :, :, :].rearrange("a p b -> p a b"),
                )
            nc.sync.dma_start(out=pb_bc, in_=pb_d[0:1, :].to_broadcast((128, C)))

        def load_x(b):
            xTb = xt_pool.tile([128, KC, N], BF16, tag="xTb")
            for kc in range(KC):
                nc.sync.dma_start(
                    out=xTb[:, kc, :],
                    in_=xT_d[kc * 128:(kc + 1) * 128, b * N:(b + 1) * N],
                )
            return xTb

        def alloc_qkT():
            return perb.tile([128, 2 * H // 2, N], BF16, tag="qkT", name="qkT")

        def alloc_vpad():
            v_pad = perb.tile([128, NIT, VP], BF16, tag="v_pad")
            ones_view = v_pad.rearrange("p a (h e) -> p (a h) e", e=HD + 1)
            nc.vector.memset(ones_view[:, :, HD:HD + 1], 1.0)
            return v_pad

        def qk_tile(qkT, xTb, rt):
            ps = mm_ps.tile([128, N], F32, tag="mm")
            for kc in range(KC):
                nc.tensor.matmul(
                    ps,
                    wqk[:, kc, rt * 128:(rt + 1) * 128],
                    xTb[:, kc, :],
                    start=(kc == 0),
                    stop=(kc == KC - 1),
                )
            nc.vector.tensor_copy(out=qkT[:, rt, :], in_=ps)

        def v_tile(v_pad, xTb, it, nh):
            ps = mm_ps.tile([128, C // 2], F32, tag="mm")
            for kc in range(KC):
                nc.tensor.matmul(
                    ps,
                    xTb[:, kc, it * 128:(it + 1) * 128],
                    wv[:, kc, nh * 384:(nh + 1) * 384],
                    start=(kc == 0),
                    stop=(kc == KC - 1),
                )
            dest = v_pad[:, it, :].rearrange("p (h e) -> p h e", e=HD + 1)
            nc.scalar.copy(
                out=dest[:, nh * 6:(nh + 1) * 6, 0:HD],
                in_=ps.rearrange("p (h e) -> p h e", e=HD),
            )

        def proj_tile(b, oT, it):
            outsb = pool_o.tile([128, C], F32, tag="outsb")
            for ct in range(2):
                ps = mm_ps.tile([128, C // 2], F32, tag="mm")
                for kc in range(KC):
                    nc.tensor.matmul(
                        ps,
                        oT[:, kc, it * 128:(it + 1) * 128],
                        wp[:, kc, ct * 384:(ct + 1) * 384],
                        start=(kc == 0),
                        stop=(kc == KC - 1),
                    )
                if PB_IS_ZERO:
                    nc.scalar.copy(out=outsb[:, ct * 384:(ct + 1) * 384], in_=ps)
                else:
                    nc.vector.tensor_add(
                        outsb[:, ct * 384:(ct + 1) * 384],
                        ps,
                        pb_bc[:, ct * 384:(ct + 1) * 384],
                    )
            nc.sync.dma_start(
                out=out_d[b * N + it * 128: b * N + (it + 1) * 128, :],
                in_=outsb,
            )

        def pair_scores(qkT, hp, jp):
            # head pair (hA even -> PE rows 0-63, hB odd -> rows 64-127):
            # their K=64 score matmuls execute concurrently on disjoint
            # row groups of the systolic array; I^T@biasT rides the same
            # PSUM accumulation; exp straight from PSUM, 1024 wide.
            rq, rk = hp, H // 2 + hp
            psA = s_ps.tile([128, 2, N], F32, tag="sTA")
            psB = s_ps.tile([128, 2, N], F32, tag="sTB")
            for jl in range(2):
                jt = jp * 2 + jl
                nc.tensor.matmul(
                    psA[:, jl, :],
                    qkT[0:64, rk, jt * 128:(jt + 1) * 128],
                    qkT[0:64, rq, :],
                    start=True,
                    stop=True,
                )
                nc.tensor.matmul(
                    psB[:, jl, :],
                    qkT[64:128, rk, jt * 128:(jt + 1) * 128],
                    qkT[64:128, rq, :],
                    start=True,
                    stop=True,
                )
            return psA, psB

        def pair_exp(jp, psA, psB, peA4, peB4):
            nc.scalar.activation(out=peA4[:, 2 * jp:2 * jp + 2, :], in_=psA,
                                 func=Exp)
            nc.scalar.activation(out=peB4[:, 2 * jp:2 * jp + 2, :], in_=psB,
                                 func=Exp)

        def head_out(oT, v_pad, h, pt4):
            po = (h % 2) * 64
            rqo = h // 2
            pso = o_ps.tile([HD + 1, N], F32, tag="oT")
            for jt in range(NJT):
                vp = v_pad[:, jt, :].rearrange("p (h e) -> p h e", e=HD + 1)
                nc.tensor.matmul(
                    pso,
                    vp[:, h, :],
                    pt4[:, jt, :],
                    start=(jt == 0),
                    stop=(jt == NJT - 1),
                )
            nc.vector.tensor_copy(out=oT[po:po + 64, rqo, :], in_=pso[0:HD, :])
            smc = pool_r.tile([65, N], F32, tag="smc")
            nc.vector.tensor_copy(out=smc[HD:HD + 1, :], in_=pso[HD:HD + 1, :])
            rcd = dram_p.tile([1, N], F32, tag="rcd")
            nc.sync.dma_start(out=rcd, in_=smc[HD:HD + 1, :])
            return rcd

        def pair_norm(oT, hp, rcdA, rcdB):
            # one [128,N] tile: 1/sums(A) on partitions 0-63, 1/sums(B) on
            # 64-127 -> a single reciprocal + two in-place multiplies.
            rqo = hp
            rcf = pool_r.tile([128, N], F32, tag="rcf")
            nc.sync.dma_start(out=rcf[0:64, :], in_=rcdA[0:1, :].to_broadcast((64, N)))
            nc.sync.dma_start(out=rcf[64:128, :], in_=rcdB[0:1, :].to_broadcast((64, N)))
            nc.vector.reciprocal_approx_fast(out=rcf, in_=rcf)
            nc.gpsimd.tensor_mul(
                oT[0:64, rqo, :], oT[0:64, rqo, :], rcf[0:64, :]
            )
            nc.gpsimd.tensor_mul(
                oT[64:128, rqo, :], oT[64:128, rqo, :], rcf[64:128, :]
            )

        # ---- software-pipelined schedule ----
        # A persistent filler queue carries the next batch's qkv tiles and
        # the previous batch's projection tiles; pairs pop fillers between
        # their score and o matmuls, and dependencies are force-drained
        # just in time.  This keeps the PE dense through the last batch.
        work = []  # list of (key, fn); key=(kind, b, a, c)

        def drain(pred):
            i = 0
            while i < len(work):
                if pred(work[i][0]):
                    work.pop(i)[1]()
                else:
                    i += 1

        def pop_one():
            if work:
                work.pop(0)[1]()

        # startup: interleave wqk/x(0) chunk loads so the first matmul can
        # begin after one chunk of each.
        xT_cur = xt_pool.tile([128, KC, N], BF16, tag="xTb", name="xT0")
        for kc in range(KC):
            nc.sync.dma_start(
                out=wqk[:, kc, :], in_=wqk_d[kc * 128:(kc + 1) * 128, :]
            )
            nc.sync.dma_start(
                out=xT_cur[:, kc, :], in_=xT_d[kc * 128:(kc + 1) * 128, 0:N]
            )
        for kc in range(KC):
            nc.sync.dma_start(out=wv[:, kc, :], in_=wv_d[kc * 128:(kc + 1) * 128, :])
        qkT_cur = alloc_qkT()
        vp_cur = alloc_vpad()
        for rt in range(12):
            qk_tile(qkT_cur, xT_cur, rt)
        load_consts_late()
        for it in range(NIT):
            for nh in range(2):
                v_tile(vp_cur, xT_cur, it, nh)

        oT_prev, b_prev = None, None
        deferred = []
        for b in range(BL):
            qkT, v_pad = qkT_cur, vp_cur
            fillers = list(deferred)
            deferred = []
            if b + 1 < BL:
                xT_nxt = load_x(b + 1)
                qkT_cur = alloc_qkT()
                vp_cur = alloc_vpad()
                if b + 1 == BL - 1:
                    # the last batch has no successor to fill its pair gaps:
                    # run only the tiles its first pairs need now, defer the
                    # rest (in dependency-safe order) as its own fillers.
                    early_rt = [0, 6, 1, 7]
                    late_rt = [2, 8, 3, 9, 4, 10, 5, 11]
                    fillers += [
                        (lambda rt=rt, q=qkT_cur, x=xT_nxt: qk_tile(q, x, rt))
                        for rt in early_rt
                    ]
                    fillers += [
                        (lambda it=it, v=vp_cur, x=xT_nxt: v_tile(v, x, it, 0))
                        for it in range(NIT)
                    ]
                    deferred += [
                        (lambda rt=rt, q=qkT_cur, x=xT_nxt: qk_tile(q, x, rt))
                        for rt in late_rt[:6]
                    ]
                    deferred += [
                        (lambda it=it, v=vp_cur, x=xT_nxt: v_tile(v, x, it, 1))
                        for it in range(2)
                    ]
                    deferred += [
                        (lambda rt=rt, q=qkT_cur, x=xT_nxt: qk_tile(q, x, rt))
                        for rt in late_rt[6:]
                    ]
                    deferred += [
                        (lambda it=it, v=vp_cur, x=xT_nxt: v_tile(v, x, it, 1))
                        for it in range(2, NIT)
                    ]
                else:
                    fillers += [
                        (lambda rt=rt, q=qkT_cur, x=xT_nxt: qk_tile(q, x, rt))
                        for rt in range(12)
                    ]
                    fillers += [
                        (lambda it=it, nh=nh, v=vp_cur, x=xT_nxt:
                         v_tile(v, x, it, nh))
                        for it in range(NIT) for nh in range(2)
                    ]
            if oT_prev is not None:
                fillers += [
                    (lambda it=it, ob=b_prev, ot=oT_prev: proj_tile(ob, ot, it))
                    for it in range(NIT)
                ]
            # spread fillers evenly over the 24 pair slots; the last batch's
            # fillers carry intra-batch dependencies, so keep their order and
            # pack them densely from the front instead.
            slots = [None] * 24
            nf = len(fillers)
            if nf:
                if b == BL - 1:
                    for i, f in enumerate(fillers[:24]):
                        slots[i] = f
                else:
                    for i, f in enumerate(fillers[:24]):
                        slots[(i * 24) // min(nf, 24)] = f
            extra = fillers[24:]

            def use(si):
                if slots[si] is not None:
                    slots[si]()

            oT = perb1.tile([128, KC, N], BF16, tag="oT")
            for hp in range(H // 2):
                peA4 = pool_p.tile([128, NJT, N], BF16, tag="peA")
                peB4 = pool_p.tile([128, NJT, N], BF16, tag="peB")
                ps01 = pair_scores(qkT, hp, 0)
                pair_exp(0, *ps01, peA4, peB4)
                use(hp * 4)
                ps23 = pair_scores(qkT, hp, 1)
                pair_exp(1, *ps23, peA4, peB4)
                use(hp * 4 + 1)
                ptA4 = pool_p.tile([128, NJT, N], BF16, tag="pTA")
                nc.vector.tensor_mul(ptA4, peA4, biasT[:, 2 * hp, :, :])
                ptB4 = pool_p.tile([128, NJT, N], BF16, tag="pTB")
                nc.vector.tensor_mul(ptB4, peB4, biasT[:, 2 * hp + 1, :, :])
                rcdA = head_out(oT, v_pad, 2 * hp, ptA4)
                use(hp * 4 + 2)
                rcdB = head_out(oT, v_pad, 2 * hp + 1, ptB4)
                use(hp * 4 + 3)
                pair_norm(oT, hp, rcdA, rcdB)
            for f in extra:
                f()
            oT_prev, b_prev = oT, b
        for it in range(NIT):
            proj_tile(b_prev, oT_prev, it)
    nc.finalize()
    return nc


def kernel(x, qkv_w, proj_w, proj_b, bias_table, _trace=False, _tmpdir=None):
    x = np.asarray(x, dtype=np.float32)
    qkv_w = np.asarray(qkv_w, dtype=np.float32)
    proj_w = np.asarray(proj_w, dtype=np.float32)
    proj_b = np.asarray(proj_b, dtype=np.float32)
    bias_table = np.asarray(bias_table, dtype=np.float32)

    # host-side layout prep (weights + bias table expansion)
    wq_scaled = qkv_w.copy()
    wq_scaled[:C] *= HD ** (-0.5)
    wqk = np.ascontiguousarray(wq_scaled[: 2 * C].T).astype(ml_dtypes.bfloat16)
    wv = np.ascontiguousarray(qkv_w[2 * C:].T).astype(ml_dtypes.bfloat16)
    wp = np.ascontiguousarray(proj_w.T).astype(ml_dtypes.bfloat16)
    ii = np.arange(N)
    idx = ii[None, :] - ii[:, None] + (N - 1)                     # [j, i]
    biasT = np.ascontiguousarray(
        np.exp(bias_table[idx].transpose(2, 0, 1).reshape(H, NJT, 128, N))
    ).astype(ml_dtypes.bfloat16)
    pb = proj_b.reshape(1, C)

    global PB_IS_ZERO
    PB_IS_ZERO = not np.any(proj_b)
    nc = _build()
    in_maps = []
    for m in range(NCORES):
        xs = x[m * BL:(m + 1) * BL]                               # [8, 512, 768]
        xT = np.ascontiguousarray(xs.transpose(2, 0, 1).reshape(C, T)).astype(ml_dtypes.bfloat16)
        in_maps.append(
            {"xT": xT, "wqk": wqk, "wv": wv, "wp": wp, "biasT": biasT, "pb": pb}
        )
    res = run_bass_kernel_spmd(
        nc, in_maps, core_ids=list(range(NCORES)), trace=_trace, tmpdir=_tmpdir
    )
    out = np.concatenate(
        [res.results[m]["out"].reshape(BL, N, C) for m in range(NCORES)], axis=0
    )
    if _trace:
        return out, res
    return out
